# revision 7
# baseline (speedup 1.0000x reference)
"""Trainium2 Bass kernel: MultiHeadAttention (GQA + RoPE + causal), 8-core SPMD.

Sharding: 8 cores = (batch B=2) x (kv-head KVH=4). Each core handles one
(b, kvh) pair: its 4 query heads (GQA group), one K head, one V head.
Per core: Q/K/V projections in transposed [d, t] layout, rotate-half RoPE
(interleaved-pair RoPE of the reference becomes rotate-half after a head-dim
permutation of the Wq/Wk columns, applied on host; attention is invariant to
a shared permutation of q/k head dims), transpose-free attention in S^T[s,t]
layout with unnormalized softmax (logits bounded, no max-subtract needed),
row-sums via ones-stationary matmuls, normalization on Y^T, row-sharded Wo
producing a partial [T, C] output. Host sums the 4 partials per batch
(the all-reduce / unshard step) and adds bo.

All matmuls bf16 with fp32 PSUM accumulation.
"""

import os
import sys

for _p in ("/opt/trn_rl_repo",):
    if _p not in sys.path and os.path.isdir(_p):
        sys.path.append(_p)

import numpy as np
import ml_dtypes

import concourse.bass as bass
import concourse.mybir as mybir
from concourse import bacc
import concourse.tile as tile
from concourse.bass_utils import run_bass_kernel_spmd

BF16 = ml_dtypes.bfloat16
AF = mybir.ActivationFunctionType
F32 = mybir.dt.float32
BF = mybir.dt.bfloat16

# Problem constants (hardcoded per contract)
B, T, C = 2, 2048, 2048
H, KVH, D = 16, 4, 128
G = H // KVH          # 4 query heads per core
SCALE = D ** -0.5
THETA = 10000.0
HALF = D // 2         # 64
P = 128               # partitions
NCB = C // P          # 16 contraction blocks
TC = 512              # t-chunk (moving free dim / psum bank)
NTC = T // TC         # 4
NSB = T // P          # 16 s-blocks
NCORES = 8

_cached = {}
last_run_info = {}


def _build_bass():
    nc = bacc.Bacc(None, target_bir_lowering=False)

    xt_d = nc.dram_tensor("xt", [NCB, P, T], BF, kind="ExternalInput")
    wq_d = nc.dram_tensor("wq", [NCB, P, G * D], BF, kind="ExternalInput")
    wk_d = nc.dram_tensor("wk", [NCB, P, D], BF, kind="ExternalInput")
    wv_d = nc.dram_tensor("wv", [NCB, P, D], BF, kind="ExternalInput")
    wo_d = nc.dram_tensor("wo", [G, P, C], BF, kind="ExternalInput")
    cos_d = nc.dram_tensor("cosb", [P, T], F32, kind="ExternalInput")
    sin_d = nc.dram_tensor("sinb", [P, T], F32, kind="ExternalInput")
    bq_d = nc.dram_tensor("bq", [G, D, 1], F32, kind="ExternalInput")
    bk_d = nc.dram_tensor("bk", [1, D, 1], F32, kind="ExternalInput")
    bv_d = nc.dram_tensor("bv", [1, D, 1], F32, kind="ExternalInput")
    tri_d = nc.dram_tensor("tri", [P, P], BF, kind="ExternalInput")
    ident_d = nc.dram_tensor("ident", [P, P], F32, kind="ExternalInput")
    out_d = nc.dram_tensor("out", [T, C], F32, kind="ExternalOutput")

    with tile.TileContext(nc) as tc:
        with (
            tc.tile_pool(name="consts", bufs=1) as consts,
            tc.tile_pool(name="wpool", bufs=1) as wpool,
            tc.tile_pool(name="qkv", bufs=1) as qkv,
            tc.tile_pool(name="psum", bufs=2, space="PSUM") as psum,
            tc.tile_pool(name="work", bufs=3) as work,
            tc.tile_pool(name="ptp", bufs=6) as ptp,
        ):
            # ---- constants ----
            cos_t = consts.tile([P, T], F32)
            sin_t = consts.tile([P, T], F32)
            tri_t = consts.tile([P, P], BF)
            ident_t = consts.tile([P, P], F32)
            bq_t = consts.tile([P, G], F32)
            bk_t = consts.tile([P, 1], F32)
            bv_t = consts.tile([P, 1], F32)
            ones_t = consts.tile([P, 1], BF)
            ones1_t = consts.tile([1, P], BF)
            nc.sync.dma_start(cos_t[:], cos_d[:, :])
            nc.sync.dma_start(sin_t[:], sin_d[:, :])
            nc.sync.dma_start(tri_t[:], tri_d[:, :])
            nc.sync.dma_start(ident_t[:], ident_d[:, :])
            for g in range(G):
                nc.sync.dma_start(bq_t[:, g : g + 1], bq_d[g])
            nc.sync.dma_start(bk_t[:], bk_d[0])
            nc.sync.dma_start(bv_t[:], bv_d[0])
            nc.vector.memset(ones_t[:], 1.0)
            nc.vector.memset(ones1_t[:], 1.0)

            # ---- weights ----
            wq_t = wpool.tile([P, NCB, G * D], BF)
            wk_t = wpool.tile([P, NCB, D], BF)
            wv_t = wpool.tile([P, NCB, D], BF)
            wo_t = wpool.tile([P, G, C], BF)
            for cb in range(NCB):
                nc.sync.dma_start(wq_t[:, cb, :], wq_d[cb])
                nc.sync.dma_start(wk_t[:, cb, :], wk_d[cb])
                nc.sync.dma_start(wv_t[:, cb, :], wv_d[cb])
            for g in range(G):
                nc.sync.dma_start(wo_t[:, g, :], wo_d[g])

            # ---- Q/K/V tensors (bf16, [d, t] layout; V as [s, d] blocks) ----
            qT = [qkv.tile([P, T], BF, name=f"qT{g}", tag=f"qT{g}") for g in range(G)]
            kT = qkv.tile([P, T], BF)
            vb = qkv.tile([P, NSB, D], BF)
            ytb = [qkv.tile([P, T], BF, name=f"yt{g}", tag=f"yt{g}") for g in range(G)]

            def project_rope(w_ap_fn, bias_ap, out_tile, tcc):
                """psum = sum_cb W[cb].T @ xt[cb, tchunk]; +bias; rotate-half RoPE -> bf16."""
                ts = slice(tcc * TC, (tcc + 1) * TC)
                ps = psum.tile([P, TC], F32, tag="proj")
                for cb in range(NCB):
                    nc.tensor.matmul(
                        ps[:], w_ap_fn(cb), xt_t[:, cb, ts],
                        start=(cb == 0), stop=(cb == NCB - 1),
                    )
                qf = work.tile([P, TC], F32, tag="qf")
                nc.scalar.activation(qf[:], ps[:], AF.Identity, bias=bias_ap)
                sw = work.tile([P, TC], F32, tag="sw")
                nc.sync.dma_start(sw[0:HALF, :], qf[HALF:P, :])
                nc.sync.dma_start(sw[HALF:P, :], qf[0:HALF, :])
                t1 = work.tile([P, TC], BF, tag="t1")
                t2 = work.tile([P, TC], BF, tag="t2")
                nc.vector.tensor_mul(t1[:], qf[:], cos_t[:, ts])
                nc.vector.tensor_mul(t2[:], sw[:], sin_t[:, ts])
                nc.vector.tensor_add(out_tile[:, ts], t1[:], t2[:])

            def project_v(tcc):
                ts = slice(tcc * TC, (tcc + 1) * TC)
                ps = psum.tile([P, TC], F32, tag="proj")
                for cb in range(NCB):
                    nc.tensor.matmul(
                        ps[:], wv_t[:, cb, :], xt_t[:, cb, ts],
                        start=(cb == 0), stop=(cb == NCB - 1),
                    )
                vf = work.tile([P, TC], F32, tag="qf")
                nc.scalar.activation(vf[:], ps[:], AF.Identity, bias=bv_t[:, 0:1])
                for j in range(TC // P):
                    tp = psum.tile([P, P], F32, tag="st")
                    nc.tensor.transpose(tp[:], vf[:, j * P : (j + 1) * P], ident_t[:])
                    nc.vector.tensor_copy(vb[:, tcc * (TC // P) + j, :], tp[:])

            with tc.tile_pool(name="xtp", bufs=1) as xtp:
                xt_t = xtp.tile([P, NCB, T], BF)
                for cb in range(NCB):
                    nc.sync.dma_start(xt_t[:, cb, :], xt_d[cb])

                # K, V first (needed by all heads), then Q per head
                for tcc in range(NTC):
                    project_rope(lambda cb: wk_t[:, cb, :], bk_t[:, 0:1], kT, tcc)
                for tcc in range(NTC):
                    project_v(tcc)
                for g in range(G):
                    for tcc in range(NTC):
                        project_rope(
                            lambda cb: wq_t[:, cb, g * D : (g + 1) * D],
                            bq_t[:, g : g + 1], qT[g], tcc,
                        )

            # ---- attention (S^T layout), per head, per t-chunk ----
            for g in range(G):
                for tcc in range(NTC):
                    ts = slice(tcc * TC, (tcc + 1) * TC)
                    nsb_c = 4 * tcc + 4      # causal s-blocks
                    yt_ps = psum.tile([P, TC], F32, tag="yt")
                    rs_ps = psum.tile([1, TC], F32, tag="norm")
                    for sb in range(nsb_c):
                        st_ps = psum.tile([P, TC], F32, tag="st")
                        nc.tensor.matmul(
                            st_ps[:], kT[:, sb * P : (sb + 1) * P], qT[g][:, ts],
                            start=True, stop=True,
                        )
                        pt = ptp.tile([P, TC], BF, tag="pt")
                        r = sb * P - tcc * TC
                        if r >= 0:
                            # diagonal block: exp only valid cols, mask wedge
                            if r > 0:
                                nc.vector.memset(pt[:, 0:r], 0.0)
                            nc.scalar.activation(pt[:, r:TC], st_ps[:, r:TC], AF.Exp, scale=SCALE)
                            nc.vector.tensor_mul(pt[:, r : r + P], pt[:, r : r + P], tri_t[:])
                        else:
                            nc.scalar.activation(pt[:], st_ps[:], AF.Exp, scale=SCALE)
                        nc.tensor.matmul(
                            yt_ps[:], vb[:, sb, :], pt[:],
                            start=(sb == 0), stop=(sb == nsb_c - 1),
                        )
                        nc.tensor.matmul(
                            rs_ps[:], ones_t[:], pt[:],
                            start=(sb == 0), stop=(sb == nsb_c - 1),
                        )
                    # normalize: ytb = yt * (1/rowsum) broadcast over partitions
                    rc = work.tile([1, TC], F32, tag="rc")
                    nc.vector.reciprocal(rc[:], rs_ps[:])
                    rcb = work.tile([1, TC], BF, tag="rcb")
                    nc.vector.tensor_copy(rcb[:], rc[:])
                    bc_ps = psum.tile([P, TC], F32, tag="norm")
                    nc.tensor.matmul(bc_ps[:], ones1_t[:], rcb[:], start=True, stop=True)
                    bc_sb = work.tile([P, TC], F32, tag="bc")
                    nc.scalar.activation(bc_sb[:], bc_ps[:], AF.Copy)
                    nc.vector.tensor_mul(ytb[g][:, ts], yt_ps[:], bc_sb[:])

            # ---- output projection: O[t, c] = sum_g Y_g @ Wo_g (partial) ----
            for tb in range(NSB):
                for cc in range(NTC):
                    o_ps = psum.tile([P, TC], F32, tag="proj")
                    for g in range(G):
                        nc.tensor.matmul(
                            o_ps[:],
                            ytb[g][:, tb * P : (tb + 1) * P],
                            wo_t[:, g, cc * TC : (cc + 1) * TC],
                            start=(g == 0), stop=(g == G - 1),
                        )
                    o_sb = work.tile([P, TC], F32, tag="osb")
                    nc.scalar.activation(o_sb[:], o_ps[:], AF.Copy)
                    nc.sync.dma_start(
                        out_d[tb * P : (tb + 1) * P, cc * TC : (cc + 1) * TC], o_sb[:]
                    )
    nc.compile()
    return nc


def _host_tables():
    perm = np.concatenate([np.arange(0, D, 2), np.arange(1, D, 2)])
    inv_freq = 1.0 / (THETA ** (np.arange(0, D, 2, dtype=np.float32) / D))
    t_idx = np.arange(T, dtype=np.float32)
    ang = t_idx[:, None] * inv_freq[None, :]          # [T, 64]
    cos_half = np.cos(ang).astype(np.float32).T       # [64, T]
    sin_half = np.sin(ang).astype(np.float32).T
    cos_b = np.concatenate([cos_half, cos_half], axis=0)       # [128, T]
    sin_b = np.concatenate([-sin_half, sin_half], axis=0)      # sign baked
    si = np.arange(P)[:, None]
    tj = np.arange(P)[None, :]
    tri = (si <= tj).astype(BF16)                      # [s, t] upper-tri incl diag
    ident = np.eye(P, dtype=np.float32)
    return perm, np.ascontiguousarray(cos_b), np.ascontiguousarray(sin_b), tri, ident


def kernel(x, Wq, bq, Wk, bk, Wv, bv, Wo, bo):
    global last_run_info
    if "nc" not in _cached:
        _cached["nc"] = _build_bass()
    nc = _cached["nc"]

    x = np.asarray(x, np.float32)
    Wq = np.asarray(Wq, np.float32)
    Wk = np.asarray(Wk, np.float32)
    Wv = np.asarray(Wv, np.float32)
    Wo = np.asarray(Wo, np.float32)
    bq = np.asarray(bq, np.float32)
    bk = np.asarray(bk, np.float32)
    bv = np.asarray(bv, np.float32)
    bo = np.asarray(bo, np.float32)

    perm, cos_b, sin_b, tri, ident = _host_tables()

    in_maps = []
    for core in range(NCORES):
        b, kvh = divmod(core, KVH)
        xt = np.ascontiguousarray(x[b].T).astype(BF16).reshape(NCB, P, T)
        qcols = np.arange(kvh * G * D, (kvh + 1) * G * D)
        wq_s = Wq[:, qcols].reshape(C, G, D)[:, :, perm].reshape(C, G * D)
        wq_s = wq_s.astype(BF16).reshape(NCB, P, G * D)
        wk_s = Wk[:, kvh * D : (kvh + 1) * D][:, perm].astype(BF16).reshape(NCB, P, D)
        wv_s = Wv[:, kvh * D : (kvh + 1) * D].astype(BF16).reshape(NCB, P, D)
        wo_s = Wo[kvh * G * D : (kvh + 1) * G * D, :].astype(BF16).reshape(G, P, C)
        bq_s = bq[qcols].reshape(G, D)[:, perm].reshape(G, D, 1).astype(np.float32)
        bk_s = bk[kvh * D : (kvh + 1) * D][perm].reshape(1, D, 1).astype(np.float32)
        bv_s = bv[kvh * D : (kvh + 1) * D].reshape(1, D, 1).astype(np.float32)
        in_maps.append({
            "xt": xt, "wq": np.ascontiguousarray(wq_s),
            "wk": np.ascontiguousarray(wk_s), "wv": np.ascontiguousarray(wv_s),
            "wo": np.ascontiguousarray(wo_s),
            "cosb": cos_b, "sinb": sin_b,
            "bq": np.ascontiguousarray(bq_s), "bk": bk_s, "bv": bv_s,
            "tri": np.ascontiguousarray(tri), "ident": ident,
        })

    try:
        res = run_bass_kernel_spmd(nc, in_maps, core_ids=list(range(NCORES)))
    except ModuleNotFoundError:
        # tracing requested but profiling hooks unavailable: run without trace
        os.environ["BASS_NEVER_TRACE"] = "1"
        res = run_bass_kernel_spmd(nc, in_maps, core_ids=list(range(NCORES)))
    last_run_info = {
        "exec_time_ns": res.exec_time_ns,
        "mean_exec_time_ns": res.mean_exec_time_ns,
        "profile_json": res.profile_json,
    }

    out = np.zeros((B, T, C), np.float32)
    for core in range(NCORES):
        b = core // KVH
        out[b] += res.results[core]["out"].astype(np.float32)
    out += bo[None, None, :]
    return out


# revision 8
# speedup vs baseline: 1.2181x; 1.2181x over previous
"""Trainium2 Bass kernel: MultiHeadAttention (GQA + RoPE + causal), 8-core SPMD.

Sharding: 8 cores = (batch B=2) x (kv-head KVH=4). Each core handles one
(b, kvh) pair: its 4 query heads (GQA group), one K head, one V head.
Per core: Q/K/V projections in transposed [d, t] layout, rotate-half RoPE
(interleaved-pair RoPE of the reference becomes rotate-half after a head-dim
permutation of the Wq/Wk columns, applied on host; attention is invariant to
a shared permutation of q/k head dims), transpose-free attention in S^T[s,t]
layout with unnormalized softmax (logits bounded, no max-subtract needed),
row-sums via ones-stationary matmuls, normalization on Y^T, row-sharded Wo
producing a partial [T, C] output. Host sums the 4 partials per batch
(the all-reduce / unshard step) and adds bo.

All matmuls bf16 with fp32 PSUM accumulation.
"""

import os
import sys

for _p in ("/opt/trn_rl_repo",):
    if _p not in sys.path and os.path.isdir(_p):
        sys.path.append(_p)

import numpy as np
import ml_dtypes

import concourse.bass as bass
import concourse.mybir as mybir
from concourse import bacc
import concourse.tile as tile
from concourse.bass_utils import run_bass_kernel_spmd

BF16 = ml_dtypes.bfloat16
AF = mybir.ActivationFunctionType
F32 = mybir.dt.float32
BF = mybir.dt.bfloat16

# Problem constants (hardcoded per contract)
B, T, C = 2, 2048, 2048
H, KVH, D = 16, 4, 128
G = H // KVH          # 4 query heads per core
SCALE = D ** -0.5
THETA = 10000.0
HALF = D // 2         # 64
P = 128               # partitions
NCB = C // P          # 16 contraction blocks
TC = 512              # t-chunk (moving free dim / psum bank)
NTC = T // TC         # 4
NSB = T // P          # 16 s-blocks
NCORES = 8

_cached = {}
last_run_info = {}


def _build_bass():
    nc = bacc.Bacc(None, target_bir_lowering=False)

    xt_d = nc.dram_tensor("xt", [NCB, P, T], BF, kind="ExternalInput")
    wq_d = nc.dram_tensor("wq", [NCB, P, G * D], BF, kind="ExternalInput")
    wk_d = nc.dram_tensor("wk", [NCB, P, D], BF, kind="ExternalInput")
    wv_d = nc.dram_tensor("wv", [NCB, P, D], BF, kind="ExternalInput")
    wo_d = nc.dram_tensor("wo", [G, P, C], BF, kind="ExternalInput")
    cos_d = nc.dram_tensor("cosb", [P, T], F32, kind="ExternalInput")
    sin_d = nc.dram_tensor("sinb", [P, T], F32, kind="ExternalInput")
    bq_d = nc.dram_tensor("bq", [G, D, 1], F32, kind="ExternalInput")
    bk_d = nc.dram_tensor("bk", [1, D, 1], F32, kind="ExternalInput")
    bv_d = nc.dram_tensor("bv", [1, D, 1], F32, kind="ExternalInput")
    tri_d = nc.dram_tensor("tri", [P, P], BF, kind="ExternalInput")
    ident_d = nc.dram_tensor("ident", [P, P], F32, kind="ExternalInput")
    out_d = nc.dram_tensor("out", [T, C], F32, kind="ExternalOutput")

    with tile.TileContext(nc) as tc:
        with (
            tc.tile_pool(name="consts", bufs=1) as consts,
            tc.tile_pool(name="wpool", bufs=1) as wpool,
            tc.tile_pool(name="qkv", bufs=1) as qkv,
            tc.tile_pool(name="psum", bufs=2, space="PSUM") as psum,
            tc.tile_pool(name="work", bufs=3) as work,
            tc.tile_pool(name="ptp", bufs=6) as ptp,
            tc.tile_pool(name="xtp", bufs=2) as xtp,
        ):
            # ---- constants ----
            cos_t = consts.tile([P, T], F32)
            sin_t = consts.tile([P, T], F32)
            tri_t = consts.tile([P, P], BF)
            ident_t = consts.tile([P, P], F32)
            bq_t = consts.tile([P, G], F32)
            bk_t = consts.tile([P, 1], F32)
            bv_t = consts.tile([P, 1], F32)
            ones_t = consts.tile([P, 1], BF)
            ones1_t = consts.tile([1, P], BF)
            nc.sync.dma_start(tri_t[:], tri_d[:, :])
            nc.sync.dma_start(ident_t[:], ident_d[:, :])
            for g in range(G):
                nc.sync.dma_start(bq_t[:, g : g + 1], bq_d[g])
            nc.sync.dma_start(bk_t[:], bk_d[0])
            nc.sync.dma_start(bv_t[:], bv_d[0])
            nc.sync.dma_start(cos_t[:], cos_d[:, :])
            nc.sync.dma_start(sin_t[:], sin_d[:, :])
            nc.vector.memset(ones_t[:], 1.0)
            nc.vector.memset(ones1_t[:], 1.0)

            # ---- weights (DMA in consumption order: wk, wv, wq, wo) ----
            wq_t = wpool.tile([P, NCB, G * D], BF)
            wk_t = wpool.tile([P, NCB, D], BF)
            wv_t = wpool.tile([P, NCB, D], BF)
            wo_t = wpool.tile([P, G, C], BF)
            for cb in range(NCB):
                nc.sync.dma_start(wk_t[:, cb, :], wk_d[cb])
            for cb in range(NCB):
                nc.sync.dma_start(wv_t[:, cb, :], wv_d[cb])
            for cb in range(NCB):
                nc.sync.dma_start(wq_t[:, cb, :], wq_d[cb])
            for g in range(G):
                nc.sync.dma_start(wo_t[:, g, :], wo_d[g])

            # ---- Q/K/V tensors (bf16, [d, t] layout; V as [s, d] blocks) ----
            qT = [qkv.tile([P, T], BF, name=f"qT{g}", tag=f"qT{g}") for g in range(G)]
            kT = qkv.tile([P, T], BF)
            vb = qkv.tile([P, NSB, D], BF)
            ytb = [qkv.tile([P, T], BF, name=f"yt{g}", tag=f"yt{g}") for g in range(G)]

            def project_rope(xt_c, w_ap_fn, bias_ap, out_tile, tcc):
                """psum = sum_cb W[cb].T @ xt[cb]; +bias; rotate-half RoPE -> bf16."""
                ts = slice(tcc * TC, (tcc + 1) * TC)
                ps = psum.tile([P, TC], F32, tag="proj")
                for cb in range(NCB):
                    nc.tensor.matmul(
                        ps[:], w_ap_fn(cb), xt_c[:, cb, :],
                        start=(cb == 0), stop=(cb == NCB - 1),
                    )
                qf = work.tile([P, TC], F32, tag="qf")
                nc.scalar.activation(qf[:], ps[:], AF.Identity, bias=bias_ap)
                sw = work.tile([P, TC], F32, tag="sw")
                nc.sync.dma_start(sw[0:HALF, :], qf[HALF:P, :])
                nc.sync.dma_start(sw[HALF:P, :], qf[0:HALF, :])
                t1 = work.tile([P, TC], BF, tag="t1")
                t2 = work.tile([P, TC], BF, tag="t2")
                nc.vector.tensor_mul(t1[:], qf[:], cos_t[:, ts])
                nc.vector.tensor_mul(t2[:], sw[:], sin_t[:, ts])
                nc.vector.tensor_add(out_tile[:, ts], t1[:], t2[:])

            def project_v(xt_c, tcc):
                ps = psum.tile([P, TC], F32, tag="proj")
                for cb in range(NCB):
                    nc.tensor.matmul(
                        ps[:], wv_t[:, cb, :], xt_c[:, cb, :],
                        start=(cb == 0), stop=(cb == NCB - 1),
                    )
                vf = work.tile([P, TC], F32, tag="qf")
                nc.scalar.activation(vf[:], ps[:], AF.Identity, bias=bv_t[:, 0:1])
                for j in range(TC // P):
                    tp = psum.tile([P, P], F32, tag="st")
                    nc.tensor.transpose(tp[:], vf[:, j * P : (j + 1) * P], ident_t[:])
                    nc.vector.tensor_copy(vb[:, tcc * (TC // P) + j, :], tp[:])

            def attn_head(g, tcc):
                """S^T attention for one head / t-chunk. AV/RS matmuls pipelined
                two s-blocks behind ST so PE never waits on the ACT exp.
                Returns a closure that emits the normalization (deferred)."""
                ts = slice(tcc * TC, (tcc + 1) * TC)
                nsb_c = 4 * tcc + 4
                yt_ps = psum.tile([P, TC], F32, tag="yt")
                rs_ps = psum.tile([1, TC], F32, tag="norm")
                pts = {}

                def emit_av(sb):
                    pt = pts.pop(sb)
                    nc.tensor.matmul(
                        yt_ps[:], vb[:, sb, :], pt[:],
                        start=(sb == 0), stop=(sb == nsb_c - 1),
                    )
                    nc.tensor.matmul(
                        rs_ps[:], ones_t[:], pt[:],
                        start=(sb == 0), stop=(sb == nsb_c - 1),
                    )

                for sb in range(nsb_c):
                    st_ps = psum.tile([P, TC], F32, tag="st")
                    nc.tensor.matmul(
                        st_ps[:], kT[:, sb * P : (sb + 1) * P], qT[g][:, ts],
                        start=True, stop=True,
                    )
                    pt = ptp.tile([P, TC], BF, tag="pt")
                    r = sb * P - tcc * TC
                    if r >= 0:
                        if r > 0:
                            nc.vector.memset(pt[:, 0:r], 0.0)
                        nc.scalar.activation(pt[:, r:TC], st_ps[:, r:TC], AF.Exp, scale=SCALE)
                        nc.vector.tensor_mul(pt[:, r : r + P], pt[:, r : r + P], tri_t[:])
                    else:
                        nc.scalar.activation(pt[:], st_ps[:], AF.Exp, scale=SCALE)
                    pts[sb] = pt
                    if sb >= 2:
                        emit_av(sb - 2)
                for sb in range(max(0, nsb_c - 2), nsb_c):
                    emit_av(sb)

                def emit_norm():
                    rc = work.tile([1, TC], F32, tag="rc")
                    nc.vector.reciprocal(rc[:], rs_ps[:])
                    rcb = work.tile([1, TC], BF, tag="rcb")
                    nc.vector.tensor_copy(rcb[:], rc[:])
                    bc_ps = psum.tile([P, TC], F32, tag="norm")
                    nc.tensor.matmul(bc_ps[:], ones1_t[:], rcb[:], start=True, stop=True)
                    bc_sb = work.tile([P, TC], F32, tag="bc")
                    nc.scalar.activation(bc_sb[:], bc_ps[:], AF.Copy)
                    nc.vector.tensor_mul(ytb[g][:, ts], yt_ps[:], bc_sb[:])

                return emit_norm

            def emit_wo(tcc):
                for tb in range(4 * tcc, 4 * tcc + 4):
                    for cc in range(NTC):
                        o_ps = psum.tile([P, TC], F32, tag="proj")
                        for g in range(G):
                            nc.tensor.matmul(
                                o_ps[:],
                                ytb[g][:, tb * P : (tb + 1) * P],
                                wo_t[:, g, cc * TC : (cc + 1) * TC],
                                start=(g == 0), stop=(g == G - 1),
                            )
                        o_sb = work.tile([P, TC], F32, tag="osb")
                        nc.scalar.activation(o_sb[:], o_ps[:], AF.Copy)
                        nc.sync.dma_start(
                            out_d[tb * P : (tb + 1) * P, cc * TC : (cc + 1) * TC], o_sb[:]
                        )

            pending_norm = None
            for tcc in range(NTC):
                ts = slice(tcc * TC, (tcc + 1) * TC)
                xt_c = xtp.tile([P, NCB, TC], BF, tag="xt")
                for cb in range(NCB):
                    nc.sync.dma_start(xt_c[:, cb, :], xt_d[cb][:, ts])
                project_rope(xt_c, lambda cb: wk_t[:, cb, :], bk_t[:, 0:1], kT, tcc)
                project_v(xt_c, tcc)
                for g in range(G):
                    project_rope(
                        xt_c,
                        lambda cb, g=g: wq_t[:, cb, g * D : (g + 1) * D],
                        bq_t[:, g : g + 1], qT[g], tcc,
                    )
                for g in range(G):
                    ncl = attn_head(g, tcc)
                    if pending_norm is not None:
                        pending_norm()
                    pending_norm = ncl
                    if g == 0 and tcc > 0:
                        emit_wo(tcc - 1)
            pending_norm()
            emit_wo(NTC - 1)
    nc.compile()
    return nc


def _host_tables():
    perm = np.concatenate([np.arange(0, D, 2), np.arange(1, D, 2)])
    inv_freq = 1.0 / (THETA ** (np.arange(0, D, 2, dtype=np.float32) / D))
    t_idx = np.arange(T, dtype=np.float32)
    ang = t_idx[:, None] * inv_freq[None, :]          # [T, 64]
    cos_half = np.cos(ang).astype(np.float32).T       # [64, T]
    sin_half = np.sin(ang).astype(np.float32).T
    cos_b = np.concatenate([cos_half, cos_half], axis=0)       # [128, T]
    sin_b = np.concatenate([-sin_half, sin_half], axis=0)      # sign baked
    si = np.arange(P)[:, None]
    tj = np.arange(P)[None, :]
    tri = (si <= tj).astype(BF16)                      # [s, t] upper-tri incl diag
    ident = np.eye(P, dtype=np.float32)
    return perm, np.ascontiguousarray(cos_b), np.ascontiguousarray(sin_b), tri, ident


def kernel(x, Wq, bq, Wk, bk, Wv, bv, Wo, bo):
    global last_run_info
    if "nc" not in _cached:
        _cached["nc"] = _build_bass()
    nc = _cached["nc"]

    x = np.asarray(x, np.float32)
    Wq = np.asarray(Wq, np.float32)
    Wk = np.asarray(Wk, np.float32)
    Wv = np.asarray(Wv, np.float32)
    Wo = np.asarray(Wo, np.float32)
    bq = np.asarray(bq, np.float32)
    bk = np.asarray(bk, np.float32)
    bv = np.asarray(bv, np.float32)
    bo = np.asarray(bo, np.float32)

    perm, cos_b, sin_b, tri, ident = _host_tables()

    in_maps = []
    for core in range(NCORES):
        b, kvh = divmod(core, KVH)
        xt = np.ascontiguousarray(x[b].T).astype(BF16).reshape(NCB, P, T)
        qcols = np.arange(kvh * G * D, (kvh + 1) * G * D)
        wq_s = Wq[:, qcols].reshape(C, G, D)[:, :, perm].reshape(C, G * D)
        wq_s = wq_s.astype(BF16).reshape(NCB, P, G * D)
        wk_s = Wk[:, kvh * D : (kvh + 1) * D][:, perm].astype(BF16).reshape(NCB, P, D)
        wv_s = Wv[:, kvh * D : (kvh + 1) * D].astype(BF16).reshape(NCB, P, D)
        wo_s = Wo[kvh * G * D : (kvh + 1) * G * D, :].astype(BF16).reshape(G, P, C)
        bq_s = bq[qcols].reshape(G, D)[:, perm].reshape(G, D, 1).astype(np.float32)
        bk_s = bk[kvh * D : (kvh + 1) * D][perm].reshape(1, D, 1).astype(np.float32)
        bv_s = bv[kvh * D : (kvh + 1) * D].reshape(1, D, 1).astype(np.float32)
        in_maps.append({
            "xt": xt, "wq": np.ascontiguousarray(wq_s),
            "wk": np.ascontiguousarray(wk_s), "wv": np.ascontiguousarray(wv_s),
            "wo": np.ascontiguousarray(wo_s),
            "cosb": cos_b, "sinb": sin_b,
            "bq": np.ascontiguousarray(bq_s), "bk": bk_s, "bv": bv_s,
            "tri": np.ascontiguousarray(tri), "ident": ident,
        })

    try:
        res = run_bass_kernel_spmd(nc, in_maps, core_ids=list(range(NCORES)))
    except ModuleNotFoundError:
        # tracing requested but profiling hooks unavailable: run without trace
        os.environ["BASS_NEVER_TRACE"] = "1"
        res = run_bass_kernel_spmd(nc, in_maps, core_ids=list(range(NCORES)))
    last_run_info = {
        "exec_time_ns": res.exec_time_ns,
        "mean_exec_time_ns": res.mean_exec_time_ns,
        "profile_json": res.profile_json,
    }

    out = np.zeros((B, T, C), np.float32)
    for core in range(NCORES):
        b = core // KVH
        out[b] += res.results[core]["out"].astype(np.float32)
    out += bo[None, None, :]
    return out


# revision 9
# speedup vs baseline: 1.3284x; 1.0905x over previous
"""Trainium2 Bass kernel: MultiHeadAttention (GQA + RoPE + causal), 8-core SPMD.

Sharding: 8 cores = (batch B=2) x (kv-head KVH=4). Each core handles one
(b, kvh) pair: its 4 query heads (GQA group), one K head, one V head.
Per core: Q/K/V projections in transposed [d, t] layout, rotate-half RoPE
(interleaved-pair RoPE of the reference becomes rotate-half after a head-dim
permutation of the Wq/Wk columns, applied on host; attention is invariant to
a shared permutation of q/k head dims), transpose-free attention in S^T[s,t]
layout with unnormalized softmax (logits bounded, no max-subtract needed),
row-sums via ones-stationary matmuls, normalization on Y^T, row-sharded Wo
producing a partial [T, C] output. Host sums the 4 partials per batch
(the all-reduce / unshard step) and adds bo.

All matmuls bf16 with fp32 PSUM accumulation.
"""

import os
import sys

for _p in ("/opt/trn_rl_repo",):
    if _p not in sys.path and os.path.isdir(_p):
        sys.path.append(_p)

import numpy as np
import ml_dtypes

import concourse.bass as bass
import concourse.mybir as mybir
from concourse import bacc
import concourse.tile as tile
from concourse.bass_utils import run_bass_kernel_spmd

BF16 = ml_dtypes.bfloat16
AF = mybir.ActivationFunctionType
F32 = mybir.dt.float32
BF = mybir.dt.bfloat16

# Problem constants (hardcoded per contract)
B, T, C = 2, 2048, 2048
H, KVH, D = 16, 4, 128
G = H // KVH          # 4 query heads per core
SCALE = D ** -0.5
THETA = 10000.0
HALF = D // 2         # 64
P = 128               # partitions
NCB = C // P          # 16 contraction blocks
TC = 512              # t-chunk (moving free dim / psum bank)
NTC = T // TC         # 4
NSB = T // P          # 16 s-blocks
NCORES = 8

_cached = {}
last_run_info = {}


def _build_bass():
    nc = bacc.Bacc(None, target_bir_lowering=False)

    xt_d = nc.dram_tensor("xt", [NCB, P, T], BF, kind="ExternalInput")
    wq_d = nc.dram_tensor("wq", [NCB, P, G * D], BF, kind="ExternalInput")
    wk_d = nc.dram_tensor("wk", [NCB, P, D], BF, kind="ExternalInput")
    wv_d = nc.dram_tensor("wv", [NCB, P, D], BF, kind="ExternalInput")
    wo_d = nc.dram_tensor("wo", [G, P, C], BF, kind="ExternalInput")
    cos_d = nc.dram_tensor("cosb", [P, T], F32, kind="ExternalInput")
    sin_d = nc.dram_tensor("sinb", [P, T], F32, kind="ExternalInput")
    bq_d = nc.dram_tensor("bq", [G, D, 1], F32, kind="ExternalInput")
    bk_d = nc.dram_tensor("bk", [1, D, 1], F32, kind="ExternalInput")
    bv_d = nc.dram_tensor("bv", [1, D, 1], F32, kind="ExternalInput")
    tri_d = nc.dram_tensor("tri", [P, P], BF, kind="ExternalInput")
    ident_d = nc.dram_tensor("ident", [P, P], F32, kind="ExternalInput")
    out_d = nc.dram_tensor("out", [T, C], F32, kind="ExternalOutput")

    with tile.TileContext(nc) as tc:
        with (
            tc.tile_pool(name="consts", bufs=1) as consts,
            tc.tile_pool(name="wpool", bufs=1) as wpool,
            tc.tile_pool(name="qkv", bufs=1) as qkv,
            tc.tile_pool(name="psum", bufs=2, space="PSUM") as psum,
            tc.tile_pool(name="work", bufs=3) as work,
            tc.tile_pool(name="ptp", bufs=6) as ptp,
            tc.tile_pool(name="xtp", bufs=2) as xtp,
        ):
            # ---- constants ----
            cos_t = consts.tile([P, T], F32)
            sin_t = consts.tile([P, T], F32)
            tri_t = consts.tile([P, P], BF)
            ident_t = consts.tile([P, P], F32)
            bq_t = consts.tile([P, G], F32)
            bk_t = consts.tile([P, 1], F32)
            bv_t = consts.tile([P, 1], F32)
            ones_t = consts.tile([P, 1], BF)
            ones1_t = consts.tile([1, P], BF)
            nc.sync.dma_start(tri_t[:], tri_d[:, :])
            nc.sync.dma_start(ident_t[:], ident_d[:, :])
            for g in range(G):
                nc.sync.dma_start(bq_t[:, g : g + 1], bq_d[g])
            nc.sync.dma_start(bk_t[:], bk_d[0])
            nc.sync.dma_start(bv_t[:], bv_d[0])
            nc.vector.memset(ones_t[:], 1.0)
            nc.vector.memset(ones1_t[:], 1.0)

            # ---- weights (DMA in consumption order: wk, wv, wq, wo) ----
            wq_t = wpool.tile([P, NCB, G * D], BF)
            wk_t = wpool.tile([P, NCB, D], BF)
            wv_t = wpool.tile([P, NCB, D], BF)
            wo_t = wpool.tile([P, G, C], BF)
            for cb in range(NCB):
                nc.sync.dma_start(wk_t[:, cb, :], wk_d[cb])

            # ---- Q/K/V tensors (bf16, [d, t] layout; V as [s, d] blocks) ----
            qT = [qkv.tile([P, T], BF, name=f"qT{g}", tag=f"qT{g}") for g in range(G)]
            kT = qkv.tile([P, T], BF)
            vb = qkv.tile([P, NSB, D], BF)
            ytb = [qkv.tile([P, T], BF, name=f"yt{g}", tag=f"yt{g}") for g in range(G)]

            def project_rope(xt_c, w_ap_fn, bias_ap, out_tile, tcc):
                """psum = sum_cb W[cb].T @ xt[cb]; +bias; rotate-half RoPE -> bf16."""
                ts = slice(tcc * TC, (tcc + 1) * TC)
                ps = psum.tile([P, TC], F32, tag="proj")
                for cb in range(NCB):
                    nc.tensor.matmul(
                        ps[:], w_ap_fn(cb), xt_c[:, cb, :],
                        start=(cb == 0), stop=(cb == NCB - 1),
                    )
                qf = work.tile([P, TC], F32, tag="qf")
                nc.scalar.activation(qf[:], ps[:], AF.Identity, bias=bias_ap)
                sw = work.tile([P, TC], F32, tag="sw")
                nc.sync.dma_start(sw[0:HALF, :], qf[HALF:P, :])
                nc.sync.dma_start(sw[HALF:P, :], qf[0:HALF, :])
                t1 = work.tile([P, TC], BF, tag="t1")
                t2 = work.tile([P, TC], BF, tag="t2")
                nc.vector.tensor_mul(t1[:], qf[:], cos_t[:, ts])
                nc.vector.tensor_mul(t2[:], sw[:], sin_t[:, ts])
                nc.vector.tensor_add(out_tile[:, ts], t1[:], t2[:])

            def project_v(xt_c, tcc):
                ps = psum.tile([P, TC], F32, tag="proj")
                for cb in range(NCB):
                    nc.tensor.matmul(
                        ps[:], wv_t[:, cb, :], xt_c[:, cb, :],
                        start=(cb == 0), stop=(cb == NCB - 1),
                    )
                vf = work.tile([P, TC], F32, tag="qf")
                nc.scalar.activation(vf[:], ps[:], AF.Identity, bias=bv_t[:, 0:1])
                for j in range(TC // P):
                    tp = psum.tile([P, P], F32, tag="st")
                    nc.tensor.transpose(tp[:], vf[:, j * P : (j + 1) * P], ident_t[:])
                    nc.vector.tensor_copy(vb[:, tcc * (TC // P) + j, :], tp[:])

            def attn_head(g, tcc):
                """S^T attention for one head / t-chunk. AV/RS matmuls pipelined
                two s-blocks behind ST so PE never waits on the ACT exp.
                Returns a closure that emits the normalization (deferred)."""
                ts = slice(tcc * TC, (tcc + 1) * TC)
                nsb_c = 4 * tcc + 4
                yt_ps = psum.tile([P, TC], F32, tag="yt")
                rs_ps = psum.tile([1, TC], F32, tag="norm")
                pts = {}

                def emit_av(sb):
                    pt = pts.pop(sb)
                    nc.tensor.matmul(
                        yt_ps[:], vb[:, sb, :], pt[:],
                        start=(sb == 0), stop=(sb == nsb_c - 1),
                    )
                    nc.tensor.matmul(
                        rs_ps[:], ones_t[:], pt[:],
                        start=(sb == 0), stop=(sb == nsb_c - 1),
                    )

                for sb in range(nsb_c):
                    st_ps = psum.tile([P, TC], F32, tag="st")
                    nc.tensor.matmul(
                        st_ps[:], kT[:, sb * P : (sb + 1) * P], qT[g][:, ts],
                        start=True, stop=True,
                    )
                    pt = ptp.tile([P, TC], BF, tag="pt")
                    r = sb * P - tcc * TC
                    if r >= 0:
                        if r > 0:
                            nc.vector.memset(pt[:, 0:r], 0.0)
                        nc.scalar.activation(pt[:, r:TC], st_ps[:, r:TC], AF.Exp, scale=SCALE)
                        nc.vector.tensor_mul(pt[:, r : r + P], pt[:, r : r + P], tri_t[:])
                    else:
                        nc.scalar.activation(pt[:], st_ps[:], AF.Exp, scale=SCALE)
                    pts[sb] = pt
                    if sb >= 2:
                        emit_av(sb - 2)
                for sb in range(max(0, nsb_c - 2), nsb_c):
                    emit_av(sb)

                def emit_norm():
                    rc = work.tile([1, TC], F32, tag="rc")
                    nc.vector.reciprocal(rc[:], rs_ps[:])
                    rcb = work.tile([1, TC], BF, tag="rcb")
                    nc.vector.tensor_copy(rcb[:], rc[:])
                    bc_ps = psum.tile([P, TC], F32, tag="norm")
                    nc.tensor.matmul(bc_ps[:], ones1_t[:], rcb[:], start=True, stop=True)
                    bc_sb = work.tile([P, TC], F32, tag="bc")
                    nc.scalar.activation(bc_sb[:], bc_ps[:], AF.Copy)
                    nc.vector.tensor_mul(ytb[g][:, ts], yt_ps[:], bc_sb[:])

                return emit_norm

            def emit_wo_tb(tcc, tb):
                for cc in range(NTC):
                    o_ps = psum.tile([P, TC], F32, tag="proj")
                    for g in range(G):
                        nc.tensor.matmul(
                            o_ps[:],
                            ytb[g][:, tb * P : (tb + 1) * P],
                            wo_t[:, g, cc * TC : (cc + 1) * TC],
                            start=(g == 0), stop=(g == G - 1),
                        )
                    o_sb = work.tile([P, TC], F32, tag="osb")
                    nc.scalar.activation(o_sb[:], o_ps[:], AF.Copy)
                    nc.sync.dma_start(
                        out_d[tb * P : (tb + 1) * P, cc * TC : (cc + 1) * TC], o_sb[:]
                    )

            pending_norm = None
            for tcc in range(NTC):
                ts = slice(tcc * TC, (tcc + 1) * TC)
                xt_c = xtp.tile([P, NCB, TC], BF, tag="xt")
                for cb in range(NCB):
                    nc.sync.dma_start(xt_c[:, cb, :], xt_d[cb][:, ts])
                if tcc == 0:
                    for cb in range(NCB):
                        nc.sync.dma_start(wv_t[:, cb, :], wv_d[cb])
                    nc.sync.dma_start(cos_t[:], cos_d[:, :])
                    nc.sync.dma_start(sin_t[:], sin_d[:, :])
                    for cb in range(NCB):
                        nc.sync.dma_start(wq_t[:, cb, :], wq_d[cb])
                elif tcc == 1:
                    for g in range(G):
                        nc.sync.dma_start(wo_t[:, g, :], wo_d[g])
                project_rope(xt_c, lambda cb: wk_t[:, cb, :], bk_t[:, 0:1], kT, tcc)
                project_v(xt_c, tcc)
                for g in range(G):
                    project_rope(
                        xt_c,
                        lambda cb, g=g: wq_t[:, cb, g * D : (g + 1) * D],
                        bq_t[:, g : g + 1], qT[g], tcc,
                    )
                for g in range(G):
                    ncl = attn_head(g, tcc)
                    if pending_norm is not None:
                        pending_norm()
                    pending_norm = ncl
                    if tcc > 0:
                        emit_wo_tb(tcc - 1, 4 * (tcc - 1) + g)
            pending_norm()
            for tb in range(4 * (NTC - 1), 4 * NTC):
                emit_wo_tb(NTC - 1, tb)
    nc.compile()
    return nc


def _host_tables():
    perm = np.concatenate([np.arange(0, D, 2), np.arange(1, D, 2)])
    inv_freq = 1.0 / (THETA ** (np.arange(0, D, 2, dtype=np.float32) / D))
    t_idx = np.arange(T, dtype=np.float32)
    ang = t_idx[:, None] * inv_freq[None, :]          # [T, 64]
    cos_half = np.cos(ang).astype(np.float32).T       # [64, T]
    sin_half = np.sin(ang).astype(np.float32).T
    cos_b = np.concatenate([cos_half, cos_half], axis=0)       # [128, T]
    sin_b = np.concatenate([-sin_half, sin_half], axis=0)      # sign baked
    si = np.arange(P)[:, None]
    tj = np.arange(P)[None, :]
    tri = (si <= tj).astype(BF16)                      # [s, t] upper-tri incl diag
    ident = np.eye(P, dtype=np.float32)
    return perm, np.ascontiguousarray(cos_b), np.ascontiguousarray(sin_b), tri, ident


def kernel(x, Wq, bq, Wk, bk, Wv, bv, Wo, bo):
    global last_run_info
    if "nc" not in _cached:
        _cached["nc"] = _build_bass()
    nc = _cached["nc"]

    x = np.asarray(x, np.float32)
    Wq = np.asarray(Wq, np.float32)
    Wk = np.asarray(Wk, np.float32)
    Wv = np.asarray(Wv, np.float32)
    Wo = np.asarray(Wo, np.float32)
    bq = np.asarray(bq, np.float32)
    bk = np.asarray(bk, np.float32)
    bv = np.asarray(bv, np.float32)
    bo = np.asarray(bo, np.float32)

    perm, cos_b, sin_b, tri, ident = _host_tables()

    in_maps = []
    for core in range(NCORES):
        b, kvh = divmod(core, KVH)
        xt = np.ascontiguousarray(x[b].T).astype(BF16).reshape(NCB, P, T)
        qcols = np.arange(kvh * G * D, (kvh + 1) * G * D)
        wq_s = Wq[:, qcols].reshape(C, G, D)[:, :, perm].reshape(C, G * D)
        wq_s = wq_s.astype(BF16).reshape(NCB, P, G * D)
        wk_s = Wk[:, kvh * D : (kvh + 1) * D][:, perm].astype(BF16).reshape(NCB, P, D)
        wv_s = Wv[:, kvh * D : (kvh + 1) * D].astype(BF16).reshape(NCB, P, D)
        wo_s = Wo[kvh * G * D : (kvh + 1) * G * D, :].astype(BF16).reshape(G, P, C)
        bq_s = bq[qcols].reshape(G, D)[:, perm].reshape(G, D, 1).astype(np.float32)
        bk_s = bk[kvh * D : (kvh + 1) * D][perm].reshape(1, D, 1).astype(np.float32)
        bv_s = bv[kvh * D : (kvh + 1) * D].reshape(1, D, 1).astype(np.float32)
        in_maps.append({
            "xt": xt, "wq": np.ascontiguousarray(wq_s),
            "wk": np.ascontiguousarray(wk_s), "wv": np.ascontiguousarray(wv_s),
            "wo": np.ascontiguousarray(wo_s),
            "cosb": cos_b, "sinb": sin_b,
            "bq": np.ascontiguousarray(bq_s), "bk": bk_s, "bv": bv_s,
            "tri": np.ascontiguousarray(tri), "ident": ident,
        })

    try:
        res = run_bass_kernel_spmd(nc, in_maps, core_ids=list(range(NCORES)))
    except ModuleNotFoundError:
        # tracing requested but profiling hooks unavailable: run without trace
        os.environ["BASS_NEVER_TRACE"] = "1"
        res = run_bass_kernel_spmd(nc, in_maps, core_ids=list(range(NCORES)))
    last_run_info = {
        "exec_time_ns": res.exec_time_ns,
        "mean_exec_time_ns": res.mean_exec_time_ns,
        "profile_json": res.profile_json,
    }

    out = np.zeros((B, T, C), np.float32)
    for core in range(NCORES):
        b = core // KVH
        out[b] += res.results[core]["out"].astype(np.float32)
    out += bo[None, None, :]
    return out


# revision 10
# speedup vs baseline: 1.4688x; 1.1057x over previous
"""Trainium2 Bass kernel: MultiHeadAttention (GQA + RoPE + causal), 8-core SPMD.

Sharding: 8 cores = (batch B=2) x (kv-head KVH=4). Each core handles one
(b, kvh) pair: its 4 query heads (GQA group), one K head, one V head.
Per core: Q/K/V projections in transposed [d, t] layout, rotate-half RoPE
(interleaved-pair RoPE of the reference becomes rotate-half after a head-dim
permutation of the Wq/Wk columns, applied on host; attention is invariant to
a shared permutation of q/k head dims), transpose-free attention in S^T[s,t]
layout with unnormalized softmax (logits bounded, no max-subtract needed),
row-sums via ones-stationary matmuls, normalization on Y^T, row-sharded Wo
producing a partial [T, C] output. Host sums the 4 partials per batch
(the all-reduce / unshard step) and adds bo.

All matmuls bf16 with fp32 PSUM accumulation.
"""

import os
import sys

for _p in ("/opt/trn_rl_repo",):
    if _p not in sys.path and os.path.isdir(_p):
        sys.path.append(_p)

import numpy as np
import ml_dtypes

import concourse.bass as bass
import concourse.mybir as mybir
from concourse import bacc
import concourse.tile as tile
from concourse.bass_utils import run_bass_kernel_spmd

BF16 = ml_dtypes.bfloat16
AF = mybir.ActivationFunctionType
F32 = mybir.dt.float32
BF = mybir.dt.bfloat16

# Problem constants (hardcoded per contract)
B, T, C = 2, 2048, 2048
H, KVH, D = 16, 4, 128
G = H // KVH          # 4 query heads per core
SCALE = D ** -0.5
THETA = 10000.0
HALF = D // 2         # 64
P = 128               # partitions
NCB = C // P          # 16 contraction blocks
TC = 512              # t-chunk (moving free dim / psum bank)
NTC = T // TC         # 4
NSB = T // P          # 16 s-blocks
NCORES = 8

_cached = {}
last_run_info = {}


def _build_bass():
    nc = bacc.Bacc(None, target_bir_lowering=False)

    xt_d = nc.dram_tensor("xt", [NCB, P, T], BF, kind="ExternalInput")
    wq_d = nc.dram_tensor("wq", [NCB, P, G * D], BF, kind="ExternalInput")
    wk_d = nc.dram_tensor("wk", [NCB, P, D], BF, kind="ExternalInput")
    wv_d = nc.dram_tensor("wv", [NCB, P, D], BF, kind="ExternalInput")
    wo_d = nc.dram_tensor("wo", [G, P, C], BF, kind="ExternalInput")
    cos_d = nc.dram_tensor("cosb", [P, T], BF, kind="ExternalInput")
    sin_d = nc.dram_tensor("sinb", [P, T], BF, kind="ExternalInput")
    bq_d = nc.dram_tensor("bq", [G, D, 1], F32, kind="ExternalInput")
    bk_d = nc.dram_tensor("bk", [1, D, 1], F32, kind="ExternalInput")
    bv_d = nc.dram_tensor("bv", [1, D, 1], F32, kind="ExternalInput")
    tri_d = nc.dram_tensor("tri", [P, P], BF, kind="ExternalInput")
    ident_d = nc.dram_tensor("ident", [P, P], F32, kind="ExternalInput")
    out_d = nc.dram_tensor("out", [T, C], F32, kind="ExternalOutput")

    with tile.TileContext(nc) as tc:
        with (
            tc.tile_pool(name="consts", bufs=1) as consts,
            tc.tile_pool(name="wpool", bufs=1) as wpool,
            tc.tile_pool(name="qkv", bufs=1) as qkv,
            tc.tile_pool(name="psum", bufs=2, space="PSUM") as psum,
            tc.tile_pool(name="work", bufs=3) as work,
            tc.tile_pool(name="ptp", bufs=8) as ptp,
            tc.tile_pool(name="xtp", bufs=2) as xtp,
        ):
            # ---- constants ----
            cos_t = consts.tile([P, T], BF)
            sin_t = consts.tile([P, T], BF)
            tri_t = consts.tile([P, P], BF)
            ident_t = consts.tile([P, P], F32)
            bq_t = consts.tile([P, G], F32)
            bk_t = consts.tile([P, 1], F32)
            bv_t = consts.tile([P, 1], F32)
            onesP_t = consts.tile([P, P], BF)
            nc.sync.dma_start(tri_t[:], tri_d[:, :])
            nc.sync.dma_start(ident_t[:], ident_d[:, :])
            for g in range(G):
                nc.sync.dma_start(bq_t[:, g : g + 1], bq_d[g])
            nc.sync.dma_start(bk_t[:], bk_d[0])
            nc.sync.dma_start(bv_t[:], bv_d[0])
            nc.vector.memset(onesP_t[:], 1.0)

            # ---- weights (DMA in consumption order: wk, wv, wq, wo) ----
            wq_t = wpool.tile([P, NCB, G * D], BF)
            wk_t = wpool.tile([P, NCB, D], BF)
            wv_t = wpool.tile([P, NCB, D], BF)
            wo_t = wpool.tile([P, G, C], BF)
            for cb in range(NCB):
                nc.sync.dma_start(wk_t[:, cb, :], wk_d[cb])

            # ---- Q/K/V tensors (bf16, [d, t] layout; V as [s, d] blocks) ----
            qT = [qkv.tile([P, T], BF, name=f"qT{g}", tag=f"qT{g}") for g in range(G)]
            kT = qkv.tile([P, T], BF)
            vb = qkv.tile([P, NSB, D], BF)
            ytb = [qkv.tile([P, T], BF, name=f"yt{g}", tag=f"yt{g}") for g in range(G)]

            def project_rope(xt_c, w_ap_fn, bias_ap, out_tile, tcc):
                """psum = sum_cb W[cb].T @ xt[cb]; +bias; rotate-half RoPE -> bf16."""
                ts = slice(tcc * TC, (tcc + 1) * TC)
                ps = psum.tile([P, TC], F32, tag="proj")
                for cb in range(NCB):
                    nc.tensor.matmul(
                        ps[:], w_ap_fn(cb), xt_c[:, cb, :],
                        start=(cb == 0), stop=(cb == NCB - 1),
                    )
                qf = work.tile([P, TC], F32, tag="qf")
                nc.scalar.activation(qf[:], ps[:], AF.Identity, bias=bias_ap)
                sw = work.tile([P, TC], F32, tag="sw")
                nc.sync.dma_start(sw[0:HALF, :], qf[HALF:P, :])
                nc.sync.dma_start(sw[HALF:P, :], qf[0:HALF, :])
                t1 = work.tile([P, TC], BF, tag="t1")
                t2 = work.tile([P, TC], BF, tag="t2")
                nc.vector.tensor_mul(t1[:], qf[:], cos_t[:, ts])
                nc.vector.tensor_mul(t2[:], sw[:], sin_t[:, ts])
                nc.vector.tensor_add(out_tile[:, ts], t1[:], t2[:])

            def project_v(xt_c, tcc):
                ps = psum.tile([P, TC], F32, tag="proj")
                for cb in range(NCB):
                    nc.tensor.matmul(
                        ps[:], wv_t[:, cb, :], xt_c[:, cb, :],
                        start=(cb == 0), stop=(cb == NCB - 1),
                    )
                vf = work.tile([P, TC], F32, tag="qf")
                nc.scalar.activation(vf[:], ps[:], AF.Identity, bias=bv_t[:, 0:1])
                for j in range(TC // P):
                    tp = psum.tile([P, P], F32, tag="st")
                    nc.tensor.transpose(tp[:], vf[:, j * P : (j + 1) * P], ident_t[:])
                    nc.vector.tensor_copy(vb[:, tcc * (TC // P) + j, :], tp[:])

            def attn_head(g, tcc):
                """S^T attention for one head / t-chunk. AV/RS matmuls pipelined
                two s-blocks behind ST so PE never waits on the ACT exp.
                Returns a closure that emits the normalization (deferred)."""
                ts = slice(tcc * TC, (tcc + 1) * TC)
                nsb_c = 4 * tcc + 4
                yt_ps = psum.tile([P, TC], F32, tag="yt")
                rs_ps = psum.tile([P, TC], F32, tag="norm")
                pts = {}

                def emit_av(sb):
                    pt = pts.pop(sb)
                    nc.tensor.matmul(
                        yt_ps[:], vb[:, sb, :], pt[:],
                        start=(sb == 0), stop=(sb == nsb_c - 1),
                    )
                    nc.tensor.matmul(
                        rs_ps[:], onesP_t[:], pt[:],
                        start=(sb == 0), stop=(sb == nsb_c - 1),
                    )

                for sb in range(nsb_c):
                    st_ps = psum.tile([P, TC], F32, tag="st")
                    nc.tensor.matmul(
                        st_ps[:], kT[:, sb * P : (sb + 1) * P], qT[g][:, ts],
                        start=True, stop=True,
                    )
                    pt = ptp.tile([P, TC], BF, tag="pt")
                    r = sb * P - tcc * TC
                    if r >= 0:
                        if r > 0:
                            nc.vector.memset(pt[:, 0:r], 0.0)
                        nc.scalar.activation(pt[:, r:TC], st_ps[:, r:TC], AF.Exp, scale=SCALE)
                        nc.vector.tensor_mul(pt[:, r : r + P], pt[:, r : r + P], tri_t[:])
                    else:
                        nc.scalar.activation(pt[:], st_ps[:], AF.Exp, scale=SCALE)
                    pts[sb] = pt
                    if sb >= 2:
                        emit_av(sb - 2)
                for sb in range(max(0, nsb_c - 2), nsb_c):
                    emit_av(sb)

                def emit_norm():
                    rb = work.tile([P, TC], F32, tag="rb")
                    nc.vector.reciprocal(rb[:], rs_ps[:])
                    nc.vector.tensor_mul(ytb[g][:, ts], yt_ps[:], rb[:])

                return emit_norm

            def emit_wo_tb(tcc, tb):
                for cc in range(NTC):
                    o_ps = psum.tile([P, TC], F32, tag="proj")
                    for g in range(G):
                        nc.tensor.matmul(
                            o_ps[:],
                            ytb[g][:, tb * P : (tb + 1) * P],
                            wo_t[:, g, cc * TC : (cc + 1) * TC],
                            start=(g == 0), stop=(g == G - 1),
                        )
                    o_sb = work.tile([P, TC], F32, tag="osb")
                    nc.scalar.activation(o_sb[:], o_ps[:], AF.Copy)
                    nc.sync.dma_start(
                        out_d[tb * P : (tb + 1) * P, cc * TC : (cc + 1) * TC], o_sb[:]
                    )

            pending_norm = None
            for tcc in range(NTC):
                ts = slice(tcc * TC, (tcc + 1) * TC)
                xt_c = xtp.tile([P, NCB, TC], BF, tag="xt")
                for cb in range(NCB):
                    nc.sync.dma_start(xt_c[:, cb, :], xt_d[cb][:, ts])
                if tcc == 0:
                    for cb in range(NCB):
                        nc.sync.dma_start(wv_t[:, cb, :], wv_d[cb])
                    nc.sync.dma_start(cos_t[:], cos_d[:, :])
                    nc.sync.dma_start(sin_t[:], sin_d[:, :])
                    for cb in range(NCB):
                        nc.sync.dma_start(wq_t[:, cb, :], wq_d[cb])
                elif tcc == 1:
                    for g in range(G):
                        nc.sync.dma_start(wo_t[:, g, :], wo_d[g])
                project_rope(xt_c, lambda cb: wk_t[:, cb, :], bk_t[:, 0:1], kT, tcc)
                project_v(xt_c, tcc)
                for g in range(G):
                    project_rope(
                        xt_c,
                        lambda cb, g=g: wq_t[:, cb, g * D : (g + 1) * D],
                        bq_t[:, g : g + 1], qT[g], tcc,
                    )
                for g in range(G):
                    ncl = attn_head(g, tcc)
                    if pending_norm is not None:
                        pending_norm()
                    pending_norm = ncl
                    if tcc > 0:
                        emit_wo_tb(tcc - 1, 4 * (tcc - 1) + g)
            pending_norm()
            for tb in range(4 * (NTC - 1), 4 * NTC):
                emit_wo_tb(NTC - 1, tb)
    nc.compile()
    return nc


def _host_tables():
    perm = np.concatenate([np.arange(0, D, 2), np.arange(1, D, 2)])
    inv_freq = 1.0 / (THETA ** (np.arange(0, D, 2, dtype=np.float32) / D))
    t_idx = np.arange(T, dtype=np.float32)
    ang = t_idx[:, None] * inv_freq[None, :]          # [T, 64]
    cos_half = np.cos(ang).astype(np.float32).T       # [64, T]
    sin_half = np.sin(ang).astype(np.float32).T
    cos_b = np.concatenate([cos_half, cos_half], axis=0)       # [128, T]
    sin_b = np.concatenate([-sin_half, sin_half], axis=0)      # sign baked
    si = np.arange(P)[:, None]
    tj = np.arange(P)[None, :]
    tri = (si <= tj).astype(BF16)                      # [s, t] upper-tri incl diag
    ident = np.eye(P, dtype=np.float32)
    return perm, np.ascontiguousarray(cos_b), np.ascontiguousarray(sin_b), tri, ident


def kernel(x, Wq, bq, Wk, bk, Wv, bv, Wo, bo):
    global last_run_info
    if "nc" not in _cached:
        _cached["nc"] = _build_bass()
    nc = _cached["nc"]

    x = np.asarray(x, np.float32)
    Wq = np.asarray(Wq, np.float32)
    Wk = np.asarray(Wk, np.float32)
    Wv = np.asarray(Wv, np.float32)
    Wo = np.asarray(Wo, np.float32)
    bq = np.asarray(bq, np.float32)
    bk = np.asarray(bk, np.float32)
    bv = np.asarray(bv, np.float32)
    bo = np.asarray(bo, np.float32)

    perm, cos_b, sin_b, tri, ident = _host_tables()
    cos_b = cos_b.astype(BF16)
    sin_b = sin_b.astype(BF16)

    in_maps = []
    for core in range(NCORES):
        b, kvh = divmod(core, KVH)
        xt = np.ascontiguousarray(x[b].T).astype(BF16).reshape(NCB, P, T)
        qcols = np.arange(kvh * G * D, (kvh + 1) * G * D)
        wq_s = Wq[:, qcols].reshape(C, G, D)[:, :, perm].reshape(C, G * D)
        wq_s = wq_s.astype(BF16).reshape(NCB, P, G * D)
        wk_s = Wk[:, kvh * D : (kvh + 1) * D][:, perm].astype(BF16).reshape(NCB, P, D)
        wv_s = Wv[:, kvh * D : (kvh + 1) * D].astype(BF16).reshape(NCB, P, D)
        wo_s = Wo[kvh * G * D : (kvh + 1) * G * D, :].astype(BF16).reshape(G, P, C)
        bq_s = bq[qcols].reshape(G, D)[:, perm].reshape(G, D, 1).astype(np.float32)
        bk_s = bk[kvh * D : (kvh + 1) * D][perm].reshape(1, D, 1).astype(np.float32)
        bv_s = bv[kvh * D : (kvh + 1) * D].reshape(1, D, 1).astype(np.float32)
        in_maps.append({
            "xt": xt, "wq": np.ascontiguousarray(wq_s),
            "wk": np.ascontiguousarray(wk_s), "wv": np.ascontiguousarray(wv_s),
            "wo": np.ascontiguousarray(wo_s),
            "cosb": cos_b, "sinb": sin_b,
            "bq": np.ascontiguousarray(bq_s), "bk": bk_s, "bv": bv_s,
            "tri": np.ascontiguousarray(tri), "ident": ident,
        })

    try:
        res = run_bass_kernel_spmd(nc, in_maps, core_ids=list(range(NCORES)))
    except ModuleNotFoundError:
        # tracing requested but profiling hooks unavailable: run without trace
        os.environ["BASS_NEVER_TRACE"] = "1"
        res = run_bass_kernel_spmd(nc, in_maps, core_ids=list(range(NCORES)))
    last_run_info = {
        "exec_time_ns": res.exec_time_ns,
        "mean_exec_time_ns": res.mean_exec_time_ns,
        "profile_json": res.profile_json,
    }

    out = np.zeros((B, T, C), np.float32)
    for core in range(NCORES):
        b = core // KVH
        out[b] += res.results[core]["out"].astype(np.float32)
    out += bo[None, None, :]
    return out


# revision 11
# speedup vs baseline: 1.4754x; 1.0044x over previous
"""Trainium2 Bass kernel: MultiHeadAttention (GQA + RoPE + causal), 8-core SPMD.

Sharding: 8 cores = (batch B=2) x (kv-head KVH=4). Each core handles one
(b, kvh) pair: its 4 query heads (GQA group), one K head, one V head.
Per core: Q/K/V projections in transposed [d, t] layout, rotate-half RoPE
(interleaved-pair RoPE of the reference becomes rotate-half after a head-dim
permutation of the Wq/Wk columns, applied on host; attention is invariant to
a shared permutation of q/k head dims), transpose-free attention in S^T[s,t]
layout with unnormalized softmax (logits bounded, no max-subtract needed),
row-sums via ones-stationary matmuls, normalization on Y^T, row-sharded Wo
producing a partial [T, C] output. Host sums the 4 partials per batch
(the all-reduce / unshard step) and adds bo.

All matmuls bf16 with fp32 PSUM accumulation.
"""

import os
import sys

for _p in ("/opt/trn_rl_repo",):
    if _p not in sys.path and os.path.isdir(_p):
        sys.path.append(_p)

import numpy as np
import ml_dtypes

import concourse.bass as bass
import concourse.mybir as mybir
from concourse import bacc
import concourse.tile as tile
from concourse.bass_utils import run_bass_kernel_spmd

BF16 = ml_dtypes.bfloat16
AF = mybir.ActivationFunctionType
F32 = mybir.dt.float32
BF = mybir.dt.bfloat16

# Problem constants (hardcoded per contract)
B, T, C = 2, 2048, 2048
H, KVH, D = 16, 4, 128
G = H // KVH          # 4 query heads per core
SCALE = D ** -0.5
THETA = 10000.0
HALF = D // 2         # 64
P = 128               # partitions
NCB = C // P          # 16 contraction blocks
TC = 512              # t-chunk (moving free dim / psum bank)
NTC = T // TC         # 4
NSB = T // P          # 16 s-blocks
NCORES = 8

_cached = {}
last_run_info = {}


def _build_bass():
    nc = bacc.Bacc(None, target_bir_lowering=False)

    xt_d = nc.dram_tensor("xt", [NCB, P, T], BF, kind="ExternalInput")
    wq_d = nc.dram_tensor("wq", [NCB, P, G * D], BF, kind="ExternalInput")
    wk_d = nc.dram_tensor("wk", [NCB, P, D], BF, kind="ExternalInput")
    wv_d = nc.dram_tensor("wv", [NCB, P, D], BF, kind="ExternalInput")
    wo_d = nc.dram_tensor("wo", [G, P, C], BF, kind="ExternalInput")
    cos_d = nc.dram_tensor("cosb", [P, T], BF, kind="ExternalInput")
    sin_d = nc.dram_tensor("sinb", [P, T], BF, kind="ExternalInput")
    bq_d = nc.dram_tensor("bq", [G, D, 1], F32, kind="ExternalInput")
    bk_d = nc.dram_tensor("bk", [1, D, 1], F32, kind="ExternalInput")
    bv_d = nc.dram_tensor("bv", [1, D, 1], F32, kind="ExternalInput")
    tri_d = nc.dram_tensor("tri", [P, P], BF, kind="ExternalInput")
    ident_d = nc.dram_tensor("ident", [P, P], F32, kind="ExternalInput")
    out_d = nc.dram_tensor("out", [T, C], F32, kind="ExternalOutput")

    with tile.TileContext(nc) as tc:
        with (
            tc.tile_pool(name="consts", bufs=1) as consts,
            tc.tile_pool(name="wpool", bufs=1) as wpool,
            tc.tile_pool(name="qkv", bufs=1) as qkv,
            tc.tile_pool(name="psum", bufs=2, space="PSUM") as psum,
            tc.tile_pool(name="work", bufs=3) as work,
            tc.tile_pool(name="ptp", bufs=8) as ptp,
            tc.tile_pool(name="xtp", bufs=2) as xtp,
            tc.tile_pool(name="osp", bufs=6) as osp,
        ):
            # ---- constants ----
            cos_t = consts.tile([P, T], BF)
            sin_t = consts.tile([P, T], BF)
            tri_t = consts.tile([P, P], BF)
            ident_t = consts.tile([P, P], F32)
            bq_t = consts.tile([P, G], F32)
            bk_t = consts.tile([P, 1], F32)
            bv_t = consts.tile([P, 1], F32)
            onesP_t = consts.tile([P, P], BF)
            nc.sync.dma_start(tri_t[:], tri_d[:, :])
            nc.sync.dma_start(ident_t[:], ident_d[:, :])
            for g in range(G):
                nc.sync.dma_start(bq_t[:, g : g + 1], bq_d[g])
            nc.sync.dma_start(bk_t[:], bk_d[0])
            nc.sync.dma_start(bv_t[:], bv_d[0])
            nc.vector.memset(onesP_t[:], 1.0)

            # ---- weights (DMA in consumption order: wk, wv, wq, wo) ----
            wq_t = wpool.tile([P, NCB, G * D], BF)
            wk_t = wpool.tile([P, NCB, D], BF)
            wv_t = wpool.tile([P, NCB, D], BF)
            wo_t = wpool.tile([P, G, C], BF)
            for cb in range(NCB):
                nc.sync.dma_start(wk_t[:, cb, :], wk_d[cb])

            # ---- Q/K/V tensors (bf16, [d, t] layout; V as [s, d] blocks) ----
            qT = [qkv.tile([P, T], BF, name=f"qT{g}", tag=f"qT{g}") for g in range(G)]
            kT = qkv.tile([P, T], BF)
            vb = qkv.tile([P, NSB, D], BF)
            ytb = [qkv.tile([P, T], BF, name=f"yt{g}", tag=f"yt{g}") for g in range(G)]

            def project_rope(xt_c, w_ap_fn, bias_ap, out_tile, tcc):
                """psum = sum_cb W[cb].T @ xt[cb]; +bias; rotate-half RoPE -> bf16."""
                ts = slice(tcc * TC, (tcc + 1) * TC)
                ps = psum.tile([P, TC], F32, tag="proj")
                for cb in range(NCB):
                    nc.tensor.matmul(
                        ps[:], w_ap_fn(cb), xt_c[:, cb, :],
                        start=(cb == 0), stop=(cb == NCB - 1),
                    )
                qf = work.tile([P, TC], F32, tag="qf")
                nc.scalar.activation(qf[:], ps[:], AF.Identity, bias=bias_ap)
                sw = work.tile([P, TC], F32, tag="sw")
                nc.gpsimd.dma_start(sw[0:HALF, :], qf[HALF:P, :])
                nc.gpsimd.dma_start(sw[HALF:P, :], qf[0:HALF, :])
                t1 = work.tile([P, TC], BF, tag="t1")
                t2 = work.tile([P, TC], BF, tag="t2")
                nc.vector.tensor_mul(t1[:], qf[:], cos_t[:, ts])
                nc.vector.tensor_mul(t2[:], sw[:], sin_t[:, ts])
                nc.vector.tensor_add(out_tile[:, ts], t1[:], t2[:])

            def project_v(xt_c, tcc):
                ps = psum.tile([P, TC], F32, tag="proj")
                for cb in range(NCB):
                    nc.tensor.matmul(
                        ps[:], wv_t[:, cb, :], xt_c[:, cb, :],
                        start=(cb == 0), stop=(cb == NCB - 1),
                    )
                vf = work.tile([P, TC], F32, tag="qf")
                nc.scalar.activation(vf[:], ps[:], AF.Identity, bias=bv_t[:, 0:1])
                for j in range(TC // P):
                    tp = psum.tile([P, P], F32, tag="st")
                    nc.tensor.transpose(tp[:], vf[:, j * P : (j + 1) * P], ident_t[:])
                    nc.vector.tensor_copy(vb[:, tcc * (TC // P) + j, :], tp[:])

            def attn_head(g, tcc):
                """S^T attention for one head / t-chunk. AV/RS matmuls pipelined
                two s-blocks behind ST so PE never waits on the ACT exp.
                Returns a closure that emits the normalization (deferred)."""
                ts = slice(tcc * TC, (tcc + 1) * TC)
                nsb_c = 4 * tcc + 4
                yt_ps = psum.tile([P, TC], F32, tag="yt")
                rs_ps = psum.tile([P, TC], F32, tag="norm")
                pts = {}

                def emit_av(sb):
                    pt = pts.pop(sb)
                    nc.tensor.matmul(
                        yt_ps[:], vb[:, sb, :], pt[:],
                        start=(sb == 0), stop=(sb == nsb_c - 1),
                    )
                    nc.tensor.matmul(
                        rs_ps[:], onesP_t[:], pt[:],
                        start=(sb == 0), stop=(sb == nsb_c - 1),
                    )

                for sb in range(nsb_c):
                    st_ps = psum.tile([P, TC], F32, tag="st")
                    nc.tensor.matmul(
                        st_ps[:], kT[:, sb * P : (sb + 1) * P], qT[g][:, ts],
                        start=True, stop=True,
                    )
                    pt = ptp.tile([P, TC], BF, tag="pt")
                    r = sb * P - tcc * TC
                    if r >= 0:
                        if r > 0:
                            nc.vector.memset(pt[:, 0:r], 0.0)
                        nc.scalar.activation(pt[:, r:TC], st_ps[:, r:TC], AF.Exp, scale=SCALE)
                        nc.vector.tensor_mul(pt[:, r : r + P], pt[:, r : r + P], tri_t[:])
                    else:
                        nc.scalar.activation(pt[:], st_ps[:], AF.Exp, scale=SCALE)
                    pts[sb] = pt
                    if sb >= 2:
                        emit_av(sb - 2)
                for sb in range(max(0, nsb_c - 2), nsb_c):
                    emit_av(sb)

                def emit_norm():
                    rb = work.tile([P, TC], F32, tag="rb")
                    nc.vector.reciprocal(rb[:], rs_ps[:])
                    nc.vector.tensor_mul(ytb[g][:, ts], yt_ps[:], rb[:])

                return emit_norm

            def emit_wo_tb(tcc, tb):
                for cc in range(NTC):
                    o_ps = psum.tile([P, TC], F32, tag="proj")
                    for g in range(G):
                        nc.tensor.matmul(
                            o_ps[:],
                            ytb[g][:, tb * P : (tb + 1) * P],
                            wo_t[:, g, cc * TC : (cc + 1) * TC],
                            start=(g == 0), stop=(g == G - 1),
                        )
                    o_sb = osp.tile([P, TC], F32, tag="osb")
                    nc.scalar.activation(o_sb[:], o_ps[:], AF.Copy)
                    nc.sync.dma_start(
                        out_d[tb * P : (tb + 1) * P, cc * TC : (cc + 1) * TC], o_sb[:]
                    )

            pending_norm = None
            for tcc in range(NTC):
                ts = slice(tcc * TC, (tcc + 1) * TC)
                xt_c = xtp.tile([P, NCB, TC], BF, tag="xt")
                for cb in range(NCB):
                    nc.sync.dma_start(xt_c[:, cb, :], xt_d[cb][:, ts])
                if tcc == 0:
                    nc.sync.dma_start(cos_t[:], cos_d[:, :])
                    nc.sync.dma_start(sin_t[:], sin_d[:, :])
                    for cb in range(NCB):
                        nc.sync.dma_start(wv_t[:, cb, :], wv_d[cb])
                    for cb in range(NCB):
                        nc.sync.dma_start(wq_t[:, cb, :], wq_d[cb])
                elif tcc == 1:
                    for g in range(G):
                        nc.sync.dma_start(wo_t[:, g, :], wo_d[g])
                project_rope(xt_c, lambda cb: wk_t[:, cb, :], bk_t[:, 0:1], kT, tcc)
                project_v(xt_c, tcc)
                for g in range(G):
                    project_rope(
                        xt_c,
                        lambda cb, g=g: wq_t[:, cb, g * D : (g + 1) * D],
                        bq_t[:, g : g + 1], qT[g], tcc,
                    )
                for g in range(G):
                    ncl = attn_head(g, tcc)
                    if pending_norm is not None:
                        pending_norm()
                    pending_norm = ncl
                    if tcc > 0:
                        emit_wo_tb(tcc - 1, 4 * (tcc - 1) + g)
            pending_norm()
            for tb in range(4 * (NTC - 1), 4 * NTC):
                emit_wo_tb(NTC - 1, tb)
    nc.compile()
    return nc


def _host_tables():
    perm = np.concatenate([np.arange(0, D, 2), np.arange(1, D, 2)])
    inv_freq = 1.0 / (THETA ** (np.arange(0, D, 2, dtype=np.float32) / D))
    t_idx = np.arange(T, dtype=np.float32)
    ang = t_idx[:, None] * inv_freq[None, :]          # [T, 64]
    cos_half = np.cos(ang).astype(np.float32).T       # [64, T]
    sin_half = np.sin(ang).astype(np.float32).T
    cos_b = np.concatenate([cos_half, cos_half], axis=0)       # [128, T]
    sin_b = np.concatenate([-sin_half, sin_half], axis=0)      # sign baked
    si = np.arange(P)[:, None]
    tj = np.arange(P)[None, :]
    tri = (si <= tj).astype(BF16)                      # [s, t] upper-tri incl diag
    ident = np.eye(P, dtype=np.float32)
    return perm, np.ascontiguousarray(cos_b), np.ascontiguousarray(sin_b), tri, ident


def kernel(x, Wq, bq, Wk, bk, Wv, bv, Wo, bo):
    global last_run_info
    if "nc" not in _cached:
        _cached["nc"] = _build_bass()
    nc = _cached["nc"]

    x = np.asarray(x, np.float32)
    Wq = np.asarray(Wq, np.float32)
    Wk = np.asarray(Wk, np.float32)
    Wv = np.asarray(Wv, np.float32)
    Wo = np.asarray(Wo, np.float32)
    bq = np.asarray(bq, np.float32)
    bk = np.asarray(bk, np.float32)
    bv = np.asarray(bv, np.float32)
    bo = np.asarray(bo, np.float32)

    perm, cos_b, sin_b, tri, ident = _host_tables()
    cos_b = cos_b.astype(BF16)
    sin_b = sin_b.astype(BF16)

    in_maps = []
    for core in range(NCORES):
        b, kvh = divmod(core, KVH)
        xt = np.ascontiguousarray(x[b].T).astype(BF16).reshape(NCB, P, T)
        qcols = np.arange(kvh * G * D, (kvh + 1) * G * D)
        wq_s = Wq[:, qcols].reshape(C, G, D)[:, :, perm].reshape(C, G * D)
        wq_s = wq_s.astype(BF16).reshape(NCB, P, G * D)
        wk_s = Wk[:, kvh * D : (kvh + 1) * D][:, perm].astype(BF16).reshape(NCB, P, D)
        wv_s = Wv[:, kvh * D : (kvh + 1) * D].astype(BF16).reshape(NCB, P, D)
        wo_s = Wo[kvh * G * D : (kvh + 1) * G * D, :].astype(BF16).reshape(G, P, C)
        bq_s = bq[qcols].reshape(G, D)[:, perm].reshape(G, D, 1).astype(np.float32)
        bk_s = bk[kvh * D : (kvh + 1) * D][perm].reshape(1, D, 1).astype(np.float32)
        bv_s = bv[kvh * D : (kvh + 1) * D].reshape(1, D, 1).astype(np.float32)
        in_maps.append({
            "xt": xt, "wq": np.ascontiguousarray(wq_s),
            "wk": np.ascontiguousarray(wk_s), "wv": np.ascontiguousarray(wv_s),
            "wo": np.ascontiguousarray(wo_s),
            "cosb": cos_b, "sinb": sin_b,
            "bq": np.ascontiguousarray(bq_s), "bk": bk_s, "bv": bv_s,
            "tri": np.ascontiguousarray(tri), "ident": ident,
        })

    try:
        res = run_bass_kernel_spmd(nc, in_maps, core_ids=list(range(NCORES)))
    except ModuleNotFoundError:
        # tracing requested but profiling hooks unavailable: run without trace
        os.environ["BASS_NEVER_TRACE"] = "1"
        res = run_bass_kernel_spmd(nc, in_maps, core_ids=list(range(NCORES)))
    last_run_info = {
        "exec_time_ns": res.exec_time_ns,
        "mean_exec_time_ns": res.mean_exec_time_ns,
        "profile_json": res.profile_json,
    }

    out = np.zeros((B, T, C), np.float32)
    for core in range(NCORES):
        b = core // KVH
        out[b] += res.results[core]["out"].astype(np.float32)
    out += bo[None, None, :]
    return out


# revision 12
# speedup vs baseline: 1.5340x; 1.0397x over previous
"""Trainium2 Bass kernel: MultiHeadAttention (GQA + RoPE + causal), 8-core SPMD.

Sharding: 8 cores = (batch B=2) x (kv-head KVH=4). Each core handles one
(b, kvh) pair: its 4 query heads (GQA group), one K head, one V head.
Per core: Q/K/V projections in transposed [d, t] layout, rotate-half RoPE
(interleaved-pair RoPE of the reference becomes rotate-half after a head-dim
permutation of the Wq/Wk columns, applied on host; attention is invariant to
a shared permutation of q/k head dims), transpose-free attention in S^T[s,t]
layout with unnormalized softmax (logits bounded, no max-subtract needed),
row-sums via ones-stationary matmuls, normalization on Y^T, row-sharded Wo
producing a partial [T, C] output. Host sums the 4 partials per batch
(the all-reduce / unshard step) and adds bo.

All matmuls bf16 with fp32 PSUM accumulation.
"""

import os
import sys

for _p in ("/opt/trn_rl_repo",):
    if _p not in sys.path and os.path.isdir(_p):
        sys.path.append(_p)

import numpy as np
import ml_dtypes

import concourse.bass as bass
import concourse.mybir as mybir
from concourse import bacc
import concourse.tile as tile
from concourse.bass_utils import run_bass_kernel_spmd

BF16 = ml_dtypes.bfloat16
AF = mybir.ActivationFunctionType
F32 = mybir.dt.float32
BF = mybir.dt.bfloat16

# Problem constants (hardcoded per contract)
B, T, C = 2, 2048, 2048
H, KVH, D = 16, 4, 128
G = H // KVH          # 4 query heads per core
SCALE = D ** -0.5
THETA = 10000.0
HALF = D // 2         # 64
P = 128               # partitions
NCB = C // P          # 16 contraction blocks
TC = 512              # t-chunk (moving free dim / psum bank)
NTC = T // TC         # 4
NSB = T // P          # 16 s-blocks
NCORES = 8

_cached = {}
last_run_info = {}


def _build_bass():
    nc = bacc.Bacc(None, target_bir_lowering=False)

    xt_d = nc.dram_tensor("xt", [P, NCB, T], BF, kind="ExternalInput")
    wq_d = nc.dram_tensor("wq", [P, NCB, G * D], BF, kind="ExternalInput")
    wk_d = nc.dram_tensor("wk", [P, NCB, D], BF, kind="ExternalInput")
    wv_d = nc.dram_tensor("wv", [P, NCB, D], BF, kind="ExternalInput")
    wo_d = nc.dram_tensor("wo", [P, G, C], BF, kind="ExternalInput")
    cos_d = nc.dram_tensor("cosb", [P, T], BF, kind="ExternalInput")
    sin_d = nc.dram_tensor("sinb", [P, T], BF, kind="ExternalInput")
    bq_d = nc.dram_tensor("bq", [G, D, 1], F32, kind="ExternalInput")
    bk_d = nc.dram_tensor("bk", [1, D, 1], F32, kind="ExternalInput")
    bv_d = nc.dram_tensor("bv", [1, D, 1], F32, kind="ExternalInput")
    tri_d = nc.dram_tensor("tri", [P, P], BF, kind="ExternalInput")
    ident_d = nc.dram_tensor("ident", [P, P], F32, kind="ExternalInput")
    out_d = nc.dram_tensor("out", [T, C], F32, kind="ExternalOutput")

    with tile.TileContext(nc) as tc:
        with (
            tc.tile_pool(name="consts", bufs=1) as consts,
            tc.tile_pool(name="wpool", bufs=1) as wpool,
            tc.tile_pool(name="qkv", bufs=1) as qkv,
            tc.tile_pool(name="psum", bufs=2, space="PSUM") as psum,
            tc.tile_pool(name="work", bufs=3) as work,
            tc.tile_pool(name="ptp", bufs=8) as ptp,
            tc.tile_pool(name="xtp", bufs=2) as xtp,
            tc.tile_pool(name="osp", bufs=3) as osp,
        ):
            # ---- constants ----
            cos_t = consts.tile([P, T], BF)
            sin_t = consts.tile([P, T], BF)
            tri_t = consts.tile([P, P], BF)
            ident_t = consts.tile([P, P], F32)
            bq_t = consts.tile([P, G], F32)
            bk_t = consts.tile([P, 1], F32)
            bv_t = consts.tile([P, 1], F32)
            onesP_t = consts.tile([P, P], BF)
            nc.sync.dma_start(tri_t[:], tri_d[:, :])
            nc.sync.dma_start(ident_t[:], ident_d[:, :])
            for g in range(G):
                nc.sync.dma_start(bq_t[:, g : g + 1], bq_d[g])
            nc.sync.dma_start(bk_t[:], bk_d[0])
            nc.sync.dma_start(bv_t[:], bv_d[0])
            nc.vector.memset(onesP_t[:], 1.0)
            scratch_t = consts.tile([P, TC], BF)
            nc.vector.memset(scratch_t[:], 0.0)
            for _w in range(24):
                wu_ps = psum.tile([P, TC], F32, tag="st")
                nc.tensor.matmul(wu_ps[:], onesP_t[:], scratch_t[:], start=True, stop=True)

            # ---- weights (DMA in consumption order: wk, wv, wq, wo) ----
            wq_t = wpool.tile([P, NCB, G * D], BF)
            wk_t = wpool.tile([P, NCB, D], BF)
            wv_t = wpool.tile([P, NCB, D], BF)
            wo_t = wpool.tile([P, G, C], BF)
            nc.sync.dma_start(wk_t[:], wk_d[:, :, :])

            # ---- Q/K/V tensors (bf16, [d, t] layout; V as [s, d] blocks) ----
            qT = [qkv.tile([P, T], BF, name=f"qT{g}", tag=f"qT{g}") for g in range(G)]
            kT = qkv.tile([P, T], BF)
            vb = qkv.tile([P, NSB, D], BF)
            ytb = [qkv.tile([P, T], BF, name=f"yt{g}", tag=f"yt{g}") for g in range(G)]

            def project_rope(xt_c, w_ap_fn, bias_ap, out_tile, tcc):
                """psum = sum_cb W[cb].T @ xt[cb]; +bias; rotate-half RoPE -> bf16."""
                ts = slice(tcc * TC, (tcc + 1) * TC)
                ps = psum.tile([P, TC], F32, tag="proj")
                for cb in range(NCB):
                    nc.tensor.matmul(
                        ps[:], w_ap_fn(cb), xt_c[:, cb, :],
                        start=(cb == 0), stop=(cb == NCB - 1),
                    )
                qf = work.tile([P, TC], F32, tag="qf")
                nc.scalar.activation(qf[:], ps[:], AF.Identity, bias=bias_ap)
                sw = work.tile([P, TC], F32, tag="sw")
                nc.gpsimd.dma_start(sw[0:HALF, :], qf[HALF:P, :])
                nc.gpsimd.dma_start(sw[HALF:P, :], qf[0:HALF, :])
                t1 = work.tile([P, TC], BF, tag="t1")
                t2 = work.tile([P, TC], BF, tag="t2")
                nc.vector.tensor_mul(t1[:], qf[:], cos_t[:, ts])
                nc.vector.tensor_mul(t2[:], sw[:], sin_t[:, ts])
                nc.vector.tensor_add(out_tile[:, ts], t1[:], t2[:])

            def project_v(xt_c, tcc):
                ps = psum.tile([P, TC], F32, tag="proj")
                for cb in range(NCB):
                    nc.tensor.matmul(
                        ps[:], wv_t[:, cb, :], xt_c[:, cb, :],
                        start=(cb == 0), stop=(cb == NCB - 1),
                    )
                vf = work.tile([P, TC], F32, tag="qf")
                nc.scalar.activation(vf[:], ps[:], AF.Identity, bias=bv_t[:, 0:1])
                for j in range(TC // P):
                    tp = psum.tile([P, P], F32, tag="st")
                    nc.tensor.transpose(tp[:], vf[:, j * P : (j + 1) * P], ident_t[:])
                    nc.vector.tensor_copy(vb[:, tcc * (TC // P) + j, :], tp[:])

            def attn_head(g, tcc):
                """S^T attention for one head / t-chunk. AV/RS matmuls pipelined
                two s-blocks behind ST so PE never waits on the ACT exp.
                Returns a closure that emits the normalization (deferred)."""
                ts = slice(tcc * TC, (tcc + 1) * TC)
                nsb_c = 4 * tcc + 4
                yt_ps = psum.tile([P, TC], F32, tag="yt")
                rs_ps = psum.tile([P, TC], F32, tag="norm")
                pts = {}

                def emit_av(sb):
                    pt = pts.pop(sb)
                    nc.tensor.matmul(
                        yt_ps[:], vb[:, sb, :], pt[:],
                        start=(sb == 0), stop=(sb == nsb_c - 1),
                    )
                    nc.tensor.matmul(
                        rs_ps[:], onesP_t[:], pt[:],
                        start=(sb == 0), stop=(sb == nsb_c - 1),
                    )

                for sb in range(nsb_c):
                    st_ps = psum.tile([P, TC], F32, tag="st")
                    nc.tensor.matmul(
                        st_ps[:], kT[:, sb * P : (sb + 1) * P], qT[g][:, ts],
                        start=True, stop=True,
                    )
                    pt = ptp.tile([P, TC], BF, tag="pt")
                    r = sb * P - tcc * TC
                    if r >= 0:
                        if r > 0:
                            nc.vector.memset(pt[:, 0:r], 0.0)
                        nc.scalar.activation(pt[:, r:TC], st_ps[:, r:TC], AF.Exp, scale=SCALE)
                        nc.vector.tensor_mul(pt[:, r : r + P], pt[:, r : r + P], tri_t[:])
                    else:
                        nc.scalar.activation(pt[:], st_ps[:], AF.Exp, scale=SCALE)
                    pts[sb] = pt
                    if sb >= 2:
                        emit_av(sb - 2)
                for sb in range(max(0, nsb_c - 2), nsb_c):
                    emit_av(sb)

                def emit_norm():
                    rb = work.tile([P, TC], F32, tag="rb")
                    nc.vector.reciprocal(rb[:], rs_ps[:])
                    nc.vector.tensor_mul(ytb[g][:, ts], yt_ps[:], rb[:])

                return emit_norm

            def emit_wo_tb(tcc, tb):
                o_sb = osp.tile([P, C], F32, tag="osb")
                for cc in range(NTC):
                    o_ps = psum.tile([P, TC], F32, tag="proj")
                    for g in range(G):
                        nc.tensor.matmul(
                            o_ps[:],
                            ytb[g][:, tb * P : (tb + 1) * P],
                            wo_t[:, g, cc * TC : (cc + 1) * TC],
                            start=(g == 0), stop=(g == G - 1),
                        )
                    nc.scalar.activation(o_sb[:, cc * TC : (cc + 1) * TC], o_ps[:], AF.Copy)
                nc.sync.dma_start(out_d[tb * P : (tb + 1) * P, :], o_sb[:])

            pending_norm = None
            for tcc in range(NTC):
                ts = slice(tcc * TC, (tcc + 1) * TC)
                xt_c = xtp.tile([P, NCB, TC], BF, tag="xt")
                for j in range(4):
                    nc.sync.dma_start(xt_c[:, 4 * j : 4 * j + 4, :],
                                      xt_d[:, 4 * j : 4 * j + 4, ts])
                if tcc == 0:
                    nc.sync.dma_start(cos_t[:], cos_d[:, :])
                    nc.sync.dma_start(sin_t[:], sin_d[:, :])
                    nc.sync.dma_start(wv_t[:], wv_d[:, :, :])
                    nc.sync.dma_start(wq_t[:, 0:8, :], wq_d[:, 0:8, :])
                    nc.sync.dma_start(wq_t[:, 8:16, :], wq_d[:, 8:16, :])
                elif tcc == 1:
                    nc.sync.dma_start(wo_t[:, 0:2, :], wo_d[:, 0:2, :])
                    nc.sync.dma_start(wo_t[:, 2:4, :], wo_d[:, 2:4, :])
                project_rope(xt_c, lambda cb: wk_t[:, cb, :], bk_t[:, 0:1], kT, tcc)
                project_v(xt_c, tcc)

                def project_q(g, tcc=tcc, xt_c=xt_c):
                    project_rope(
                        xt_c,
                        lambda cb: wq_t[:, cb, g * D : (g + 1) * D],
                        bq_t[:, g : g + 1], qT[g], tcc,
                    )

                project_q(0)
                project_q(1)
                for g in range(G):
                    if g + 2 < G:
                        project_q(g + 2)
                    ncl = attn_head(g, tcc)
                    if pending_norm is not None:
                        pending_norm()
                    pending_norm = ncl
                    if tcc > 0:
                        emit_wo_tb(tcc - 1, 4 * (tcc - 1) + g)
            pending_norm()
            for tb in range(4 * (NTC - 1), 4 * NTC):
                emit_wo_tb(NTC - 1, tb)
    nc.compile()
    return nc


def _host_tables():
    perm = np.concatenate([np.arange(0, D, 2), np.arange(1, D, 2)])
    inv_freq = 1.0 / (THETA ** (np.arange(0, D, 2, dtype=np.float32) / D))
    t_idx = np.arange(T, dtype=np.float32)
    ang = t_idx[:, None] * inv_freq[None, :]          # [T, 64]
    cos_half = np.cos(ang).astype(np.float32).T       # [64, T]
    sin_half = np.sin(ang).astype(np.float32).T
    cos_b = np.concatenate([cos_half, cos_half], axis=0)       # [128, T]
    sin_b = np.concatenate([-sin_half, sin_half], axis=0)      # sign baked
    si = np.arange(P)[:, None]
    tj = np.arange(P)[None, :]
    tri = (si <= tj).astype(BF16)                      # [s, t] upper-tri incl diag
    ident = np.eye(P, dtype=np.float32)
    return perm, np.ascontiguousarray(cos_b), np.ascontiguousarray(sin_b), tri, ident


def kernel(x, Wq, bq, Wk, bk, Wv, bv, Wo, bo):
    global last_run_info
    if "nc" not in _cached:
        _cached["nc"] = _build_bass()
    nc = _cached["nc"]

    x = np.asarray(x, np.float32)
    Wq = np.asarray(Wq, np.float32)
    Wk = np.asarray(Wk, np.float32)
    Wv = np.asarray(Wv, np.float32)
    Wo = np.asarray(Wo, np.float32)
    bq = np.asarray(bq, np.float32)
    bk = np.asarray(bk, np.float32)
    bv = np.asarray(bv, np.float32)
    bo = np.asarray(bo, np.float32)

    perm, cos_b, sin_b, tri, ident = _host_tables()
    cos_b = cos_b.astype(BF16)
    sin_b = sin_b.astype(BF16)

    in_maps = []
    for core in range(NCORES):
        b, kvh = divmod(core, KVH)
        xt = np.ascontiguousarray(
            x[b].T.astype(BF16).reshape(NCB, P, T).transpose(1, 0, 2))
        qcols = np.arange(kvh * G * D, (kvh + 1) * G * D)
        wq_s = Wq[:, qcols].reshape(C, G, D)[:, :, perm].reshape(C, G * D)
        wq_s = np.ascontiguousarray(
            wq_s.astype(BF16).reshape(NCB, P, G * D).transpose(1, 0, 2))
        wk_s = np.ascontiguousarray(
            Wk[:, kvh * D : (kvh + 1) * D][:, perm].astype(BF16).reshape(NCB, P, D).transpose(1, 0, 2))
        wv_s = np.ascontiguousarray(
            Wv[:, kvh * D : (kvh + 1) * D].astype(BF16).reshape(NCB, P, D).transpose(1, 0, 2))
        wo_s = np.ascontiguousarray(
            Wo[kvh * G * D : (kvh + 1) * G * D, :].astype(BF16).reshape(G, P, C).transpose(1, 0, 2))
        bq_s = bq[qcols].reshape(G, D)[:, perm].reshape(G, D, 1).astype(np.float32)
        bk_s = bk[kvh * D : (kvh + 1) * D][perm].reshape(1, D, 1).astype(np.float32)
        bv_s = bv[kvh * D : (kvh + 1) * D].reshape(1, D, 1).astype(np.float32)
        in_maps.append({
            "xt": xt, "wq": np.ascontiguousarray(wq_s),
            "wk": np.ascontiguousarray(wk_s), "wv": np.ascontiguousarray(wv_s),
            "wo": np.ascontiguousarray(wo_s),
            "cosb": cos_b, "sinb": sin_b,
            "bq": np.ascontiguousarray(bq_s), "bk": bk_s, "bv": bv_s,
            "tri": np.ascontiguousarray(tri), "ident": ident,
        })

    try:
        res = run_bass_kernel_spmd(nc, in_maps, core_ids=list(range(NCORES)))
    except ModuleNotFoundError:
        # tracing requested but profiling hooks unavailable: run without trace
        os.environ["BASS_NEVER_TRACE"] = "1"
        res = run_bass_kernel_spmd(nc, in_maps, core_ids=list(range(NCORES)))
    last_run_info = {
        "exec_time_ns": res.exec_time_ns,
        "mean_exec_time_ns": res.mean_exec_time_ns,
        "profile_json": res.profile_json,
    }

    out = np.zeros((B, T, C), np.float32)
    for core in range(NCORES):
        b = core // KVH
        out[b] += res.results[core]["out"].astype(np.float32)
    out += bo[None, None, :]
    return out


# revision 13
# speedup vs baseline: 1.5839x; 1.0326x over previous
"""Trainium2 Bass kernel: MultiHeadAttention (GQA + RoPE + causal), 8-core SPMD.

Sharding: 8 cores = (batch B=2) x (kv-head KVH=4). Each core handles one
(b, kvh) pair: its 4 query heads (GQA group), one K head, one V head.
Per core: Q/K/V projections in transposed [d, t] layout, rotate-half RoPE
(interleaved-pair RoPE of the reference becomes rotate-half after a head-dim
permutation of the Wq/Wk columns, applied on host; attention is invariant to
a shared permutation of q/k head dims), transpose-free attention in S^T[s,t]
layout with unnormalized softmax (logits bounded, no max-subtract needed),
row-sums via ones-stationary matmuls, normalization on Y^T, row-sharded Wo
producing a partial [T, C] output. Host sums the 4 partials per batch
(the all-reduce / unshard step) and adds bo.

All matmuls bf16 with fp32 PSUM accumulation.
"""

import os
import sys

for _p in ("/opt/trn_rl_repo",):
    if _p not in sys.path and os.path.isdir(_p):
        sys.path.append(_p)

import numpy as np
import ml_dtypes

import concourse.bass as bass
import concourse.mybir as mybir
from concourse import bacc
import concourse.tile as tile
from concourse.bass_utils import run_bass_kernel_spmd

BF16 = ml_dtypes.bfloat16
AF = mybir.ActivationFunctionType
F32 = mybir.dt.float32
BF = mybir.dt.bfloat16

# Problem constants (hardcoded per contract)
B, T, C = 2, 2048, 2048
H, KVH, D = 16, 4, 128
G = H // KVH          # 4 query heads per core
SCALE = D ** -0.5
THETA = 10000.0
HALF = D // 2         # 64
P = 128               # partitions
NCB = C // P          # 16 contraction blocks
TC = 512              # t-chunk (moving free dim / psum bank)
NTC = T // TC         # 4
NSB = T // P          # 16 s-blocks
NCORES = 8

_cached = {}
last_run_info = {}


def _build_bass():
    nc = bacc.Bacc(None, target_bir_lowering=False)

    xt_d = nc.dram_tensor("xt", [P, NCB, T], BF, kind="ExternalInput")
    wq_d = nc.dram_tensor("wq", [P, NCB, G * D], BF, kind="ExternalInput")
    wk_d = nc.dram_tensor("wk", [P, NCB, D], BF, kind="ExternalInput")
    wv_d = nc.dram_tensor("wv", [P, NCB, D], BF, kind="ExternalInput")
    wo_d = nc.dram_tensor("wo", [P, G, C], BF, kind="ExternalInput")
    cos_d = nc.dram_tensor("cosb", [P, T], BF, kind="ExternalInput")
    sin_d = nc.dram_tensor("sinb", [P, T], BF, kind="ExternalInput")
    bq_d = nc.dram_tensor("bq", [G, D, 1], F32, kind="ExternalInput")
    bk_d = nc.dram_tensor("bk", [1, D, 1], F32, kind="ExternalInput")
    bv_d = nc.dram_tensor("bv", [1, D, 1], F32, kind="ExternalInput")
    tri_d = nc.dram_tensor("tri", [P, P], BF, kind="ExternalInput")
    ident_d = nc.dram_tensor("ident", [P, P], F32, kind="ExternalInput")
    out_d = nc.dram_tensor("out", [T, C], F32, kind="ExternalOutput")

    with tile.TileContext(nc) as tc:
        with (
            tc.tile_pool(name="consts", bufs=1) as consts,
            tc.tile_pool(name="wpool", bufs=1) as wpool,
            tc.tile_pool(name="qkv", bufs=1) as qkv,
            tc.tile_pool(name="psum", bufs=2, space="PSUM") as psum,
            tc.tile_pool(name="work", bufs=3) as work,
            tc.tile_pool(name="ptp", bufs=8) as ptp,
            tc.tile_pool(name="xtp", bufs=3) as xtp,
            tc.tile_pool(name="osp", bufs=3) as osp,
        ):
            # ---- constants ----
            cos_t = consts.tile([P, T], BF)
            sin_t = consts.tile([P, T], BF)
            tri_t = consts.tile([P, P], BF)
            ident_t = consts.tile([P, P], F32)
            bq_t = consts.tile([P, G], F32)
            bk_t = consts.tile([P, 1], F32)
            bv_t = consts.tile([P, 1], F32)
            onesP_t = consts.tile([P, P], BF)
            nc.sync.dma_start(tri_t[:], tri_d[:, :])
            nc.sync.dma_start(ident_t[:], ident_d[:, :])
            for g in range(G):
                nc.sync.dma_start(bq_t[:, g : g + 1], bq_d[g])
            nc.sync.dma_start(bk_t[:], bk_d[0])
            nc.sync.dma_start(bv_t[:], bv_d[0])
            nc.vector.memset(onesP_t[:], 1.0)
            scratch_t = consts.tile([P, TC], BF)
            nc.vector.memset(scratch_t[:], 0.0)
            for _w in range(24):
                wu_ps = psum.tile([P, TC], F32, tag="st")
                nc.tensor.matmul(wu_ps[:], onesP_t[:], scratch_t[:], start=True, stop=True)

            # ---- weights (DMA in consumption order: wk, wv, wq, wo) ----
            wq_t = wpool.tile([P, NCB, G * D], BF)
            wk_t = wpool.tile([P, NCB, D], BF)
            wv_t = wpool.tile([P, NCB, D], BF)
            wo_t = wpool.tile([P, G, C], BF)
            nc.sync.dma_start(wk_t[:], wk_d[:, :, :])

            # ---- Q/K/V tensors (bf16, [d, t] layout; V as [s, d] blocks) ----
            qT = [qkv.tile([P, T], BF, name=f"qT{g}", tag=f"qT{g}") for g in range(G)]
            kT = qkv.tile([P, T], BF)
            vb = qkv.tile([P, NSB, D], BF)
            ytb = [qkv.tile([P, T], BF, name=f"yt{g}", tag=f"yt{g}") for g in range(G)]

            def project_rope(xt_c, w_ap_fn, bias_ap, out_tile, tcc):
                """psum = sum_cb W[cb].T @ xt[cb]; +bias; rotate-half RoPE -> bf16."""
                ts = slice(tcc * TC, (tcc + 1) * TC)
                ps = psum.tile([P, TC], F32, tag="proj")
                for cb in range(NCB):
                    nc.tensor.matmul(
                        ps[:], w_ap_fn(cb), xt_c[:, cb, :],
                        start=(cb == 0), stop=(cb == NCB - 1),
                    )
                qf = work.tile([P, TC], F32, tag="qf")
                nc.scalar.activation(qf[:], ps[:], AF.Identity, bias=bias_ap)
                sw = work.tile([P, TC], F32, tag="sw")
                nc.gpsimd.dma_start(sw[0:HALF, :], qf[HALF:P, :])
                nc.gpsimd.dma_start(sw[HALF:P, :], qf[0:HALF, :])
                t1 = work.tile([P, TC], BF, tag="t1")
                t2 = work.tile([P, TC], BF, tag="t2")
                nc.vector.tensor_mul(t1[:], qf[:], cos_t[:, ts])
                nc.vector.tensor_mul(t2[:], sw[:], sin_t[:, ts])
                nc.vector.tensor_add(out_tile[:, ts], t1[:], t2[:])

            def project_v(xt_c, tcc):
                ps = psum.tile([P, TC], F32, tag="proj")
                for cb in range(NCB):
                    nc.tensor.matmul(
                        ps[:], wv_t[:, cb, :], xt_c[:, cb, :],
                        start=(cb == 0), stop=(cb == NCB - 1),
                    )
                vf = work.tile([P, TC], F32, tag="qf")
                nc.scalar.activation(vf[:], ps[:], AF.Identity, bias=bv_t[:, 0:1])
                for j in range(TC // P):
                    tp = psum.tile([P, P], F32, tag="st")
                    nc.tensor.transpose(tp[:], vf[:, j * P : (j + 1) * P], ident_t[:])
                    nc.vector.tensor_copy(vb[:, tcc * (TC // P) + j, :], tp[:])

            def attn_head(g, tcc):
                """S^T attention for one head / t-chunk. AV/RS matmuls pipelined
                two s-blocks behind ST so PE never waits on the ACT exp.
                Returns a closure that emits the normalization (deferred)."""
                ts = slice(tcc * TC, (tcc + 1) * TC)
                nsb_c = 4 * tcc + 4
                yt_ps = psum.tile([P, TC], F32, tag="yt")
                rs_ps = psum.tile([P, TC], F32, tag="norm")
                pts = {}

                def emit_av(sb):
                    pt = pts.pop(sb)
                    nc.tensor.matmul(
                        yt_ps[:], vb[:, sb, :], pt[:],
                        start=(sb == 0), stop=(sb == nsb_c - 1),
                    )
                    nc.tensor.matmul(
                        rs_ps[:], onesP_t[:], pt[:],
                        start=(sb == 0), stop=(sb == nsb_c - 1),
                    )

                for sb in range(nsb_c):
                    st_ps = psum.tile([P, TC], F32, tag="st")
                    nc.tensor.matmul(
                        st_ps[:], kT[:, sb * P : (sb + 1) * P], qT[g][:, ts],
                        start=True, stop=True,
                    )
                    pt = ptp.tile([P, TC], BF, tag="pt")
                    r = sb * P - tcc * TC
                    if r >= 0:
                        if r > 0:
                            nc.vector.memset(pt[:, 0:r], 0.0)
                        nc.scalar.activation(pt[:, r:TC], st_ps[:, r:TC], AF.Exp, scale=SCALE)
                        nc.vector.tensor_mul(pt[:, r : r + P], pt[:, r : r + P], tri_t[:])
                    else:
                        nc.scalar.activation(pt[:], st_ps[:], AF.Exp, scale=SCALE)
                    pts[sb] = pt
                    if sb >= 2:
                        emit_av(sb - 2)
                for sb in range(max(0, nsb_c - 2), nsb_c):
                    emit_av(sb)

                def emit_norm():
                    rb = work.tile([P, TC], F32, tag="rb")
                    nc.vector.reciprocal(rb[:], rs_ps[:])
                    nc.vector.tensor_mul(ytb[g][:, ts], yt_ps[:], rb[:])

                return emit_norm

            def emit_wo_tb(tcc, tb):
                o_sb = osp.tile([P, C], F32, tag="osb")
                for cc in range(NTC):
                    o_ps = psum.tile([P, TC], F32, tag="proj")
                    for g in range(G):
                        nc.tensor.matmul(
                            o_ps[:],
                            ytb[g][:, tb * P : (tb + 1) * P],
                            wo_t[:, g, cc * TC : (cc + 1) * TC],
                            start=(g == 0), stop=(g == G - 1),
                        )
                    nc.scalar.activation(o_sb[:, cc * TC : (cc + 1) * TC], o_ps[:], AF.Copy)
                nc.sync.dma_start(out_d[tb * P : (tb + 1) * P, :], o_sb[:])

            def load_xt(tcc):
                ts = slice(tcc * TC, (tcc + 1) * TC)
                xt_c = xtp.tile([P, NCB, TC], BF, tag="xt")
                for j in range(4):
                    nc.sync.dma_start(xt_c[:, 4 * j : 4 * j + 4, :],
                                      xt_d[:, 4 * j : 4 * j + 4, ts])
                return xt_c

            def proj_chunks(tcc, xt_c):
                fns = [
                    lambda: project_rope(xt_c, lambda cb: wk_t[:, cb, :],
                                         bk_t[:, 0:1], kT, tcc),
                    lambda: project_v(xt_c, tcc),
                ]
                for g in range(G):
                    fns.append(lambda g=g: project_rope(
                        xt_c,
                        lambda cb: wq_t[:, cb, g * D : (g + 1) * D],
                        bq_t[:, g : g + 1], qT[g], tcc,
                    ))
                return fns

            pending_norm = None
            # prologue: t-chunk 0 input DMA + weight DMAs + projections
            xt_c = load_xt(0)
            nc.sync.dma_start(cos_t[:], cos_d[:, :])
            nc.sync.dma_start(sin_t[:], sin_d[:, :])
            nc.sync.dma_start(wv_t[:], wv_d[:, :, :])
            nc.sync.dma_start(wq_t[:, 0:8, :], wq_d[:, 0:8, :])
            nc.sync.dma_start(wq_t[:, 8:16, :], wq_d[:, 8:16, :])
            for f in proj_chunks(0, xt_c):
                f()
            for tcc in range(NTC):
                if tcc == 0:
                    nc.sync.dma_start(wo_t[:, 0:2, :], wo_d[:, 0:2, :])
                    nc.sync.dma_start(wo_t[:, 2:4, :], wo_d[:, 2:4, :])
                chunks_next = []
                if tcc + 1 < NTC:
                    xt_next = load_xt(tcc + 1)
                    chunks_next = proj_chunks(tcc + 1, xt_next)
                for g in range(G):
                    for _ in range(2):
                        if chunks_next:
                            chunks_next.pop(0)()
                    ncl = attn_head(g, tcc)
                    if pending_norm is not None:
                        pending_norm()
                    pending_norm = ncl
                    if tcc > 0:
                        emit_wo_tb(tcc - 1, 4 * (tcc - 1) + g)
            pending_norm()
            for tb in range(4 * (NTC - 1), 4 * NTC):
                emit_wo_tb(NTC - 1, tb)
    nc.compile()
    return nc


def _host_tables():
    perm = np.concatenate([np.arange(0, D, 2), np.arange(1, D, 2)])
    inv_freq = 1.0 / (THETA ** (np.arange(0, D, 2, dtype=np.float32) / D))
    t_idx = np.arange(T, dtype=np.float32)
    ang = t_idx[:, None] * inv_freq[None, :]          # [T, 64]
    cos_half = np.cos(ang).astype(np.float32).T       # [64, T]
    sin_half = np.sin(ang).astype(np.float32).T
    cos_b = np.concatenate([cos_half, cos_half], axis=0)       # [128, T]
    sin_b = np.concatenate([-sin_half, sin_half], axis=0)      # sign baked
    si = np.arange(P)[:, None]
    tj = np.arange(P)[None, :]
    tri = (si <= tj).astype(BF16)                      # [s, t] upper-tri incl diag
    ident = np.eye(P, dtype=np.float32)
    return perm, np.ascontiguousarray(cos_b), np.ascontiguousarray(sin_b), tri, ident


def kernel(x, Wq, bq, Wk, bk, Wv, bv, Wo, bo):
    global last_run_info
    if "nc" not in _cached:
        _cached["nc"] = _build_bass()
    nc = _cached["nc"]

    x = np.asarray(x, np.float32)
    Wq = np.asarray(Wq, np.float32)
    Wk = np.asarray(Wk, np.float32)
    Wv = np.asarray(Wv, np.float32)
    Wo = np.asarray(Wo, np.float32)
    bq = np.asarray(bq, np.float32)
    bk = np.asarray(bk, np.float32)
    bv = np.asarray(bv, np.float32)
    bo = np.asarray(bo, np.float32)

    perm, cos_b, sin_b, tri, ident = _host_tables()
    cos_b = cos_b.astype(BF16)
    sin_b = sin_b.astype(BF16)

    in_maps = []
    for core in range(NCORES):
        b, kvh = divmod(core, KVH)
        xt = np.ascontiguousarray(
            x[b].T.astype(BF16).reshape(NCB, P, T).transpose(1, 0, 2))
        qcols = np.arange(kvh * G * D, (kvh + 1) * G * D)
        wq_s = Wq[:, qcols].reshape(C, G, D)[:, :, perm].reshape(C, G * D)
        wq_s = np.ascontiguousarray(
            wq_s.astype(BF16).reshape(NCB, P, G * D).transpose(1, 0, 2))
        wk_s = np.ascontiguousarray(
            Wk[:, kvh * D : (kvh + 1) * D][:, perm].astype(BF16).reshape(NCB, P, D).transpose(1, 0, 2))
        wv_s = np.ascontiguousarray(
            Wv[:, kvh * D : (kvh + 1) * D].astype(BF16).reshape(NCB, P, D).transpose(1, 0, 2))
        wo_s = np.ascontiguousarray(
            Wo[kvh * G * D : (kvh + 1) * G * D, :].astype(BF16).reshape(G, P, C).transpose(1, 0, 2))
        bq_s = bq[qcols].reshape(G, D)[:, perm].reshape(G, D, 1).astype(np.float32)
        bk_s = bk[kvh * D : (kvh + 1) * D][perm].reshape(1, D, 1).astype(np.float32)
        bv_s = bv[kvh * D : (kvh + 1) * D].reshape(1, D, 1).astype(np.float32)
        in_maps.append({
            "xt": xt, "wq": np.ascontiguousarray(wq_s),
            "wk": np.ascontiguousarray(wk_s), "wv": np.ascontiguousarray(wv_s),
            "wo": np.ascontiguousarray(wo_s),
            "cosb": cos_b, "sinb": sin_b,
            "bq": np.ascontiguousarray(bq_s), "bk": bk_s, "bv": bv_s,
            "tri": np.ascontiguousarray(tri), "ident": ident,
        })

    try:
        res = run_bass_kernel_spmd(nc, in_maps, core_ids=list(range(NCORES)))
    except ModuleNotFoundError:
        # tracing requested but profiling hooks unavailable: run without trace
        os.environ["BASS_NEVER_TRACE"] = "1"
        res = run_bass_kernel_spmd(nc, in_maps, core_ids=list(range(NCORES)))
    last_run_info = {
        "exec_time_ns": res.exec_time_ns,
        "mean_exec_time_ns": res.mean_exec_time_ns,
        "profile_json": res.profile_json,
    }

    out = np.zeros((B, T, C), np.float32)
    for core in range(NCORES):
        b = core // KVH
        out[b] += res.results[core]["out"].astype(np.float32)
    out += bo[None, None, :]
    return out


# revision 14
# speedup vs baseline: 1.5874x; 1.0022x over previous
"""Trainium2 Bass kernel: MultiHeadAttention (GQA + RoPE + causal), 8-core SPMD.

Sharding: 8 cores = (batch B=2) x (kv-head KVH=4). Each core handles one
(b, kvh) pair: its 4 query heads (GQA group), one K head, one V head.
Per core: Q/K/V projections in transposed [d, t] layout, rotate-half RoPE
(interleaved-pair RoPE of the reference becomes rotate-half after a head-dim
permutation of the Wq/Wk columns, applied on host; attention is invariant to
a shared permutation of q/k head dims), transpose-free attention in S^T[s,t]
layout with unnormalized softmax (logits bounded, no max-subtract needed),
row-sums via ones-stationary matmuls, normalization on Y^T, row-sharded Wo
producing a partial [T, C] output. Host sums the 4 partials per batch
(the all-reduce / unshard step) and adds bo.

All matmuls bf16 with fp32 PSUM accumulation.
"""

import os
import sys

for _p in ("/opt/trn_rl_repo",):
    if _p not in sys.path and os.path.isdir(_p):
        sys.path.append(_p)

import numpy as np
import ml_dtypes

import concourse.bass as bass
import concourse.mybir as mybir
from concourse import bacc
import concourse.tile as tile
from concourse.bass_utils import run_bass_kernel_spmd

BF16 = ml_dtypes.bfloat16
AF = mybir.ActivationFunctionType
F32 = mybir.dt.float32
BF = mybir.dt.bfloat16

# Problem constants (hardcoded per contract)
B, T, C = 2, 2048, 2048
H, KVH, D = 16, 4, 128
G = H // KVH          # 4 query heads per core
SCALE = D ** -0.5
THETA = 10000.0
HALF = D // 2         # 64
P = 128               # partitions
NCB = C // P          # 16 contraction blocks
TC = 512              # t-chunk (moving free dim / psum bank)
NTC = T // TC         # 4
NSB = T // P          # 16 s-blocks
NCORES = 8

_cached = {}
last_run_info = {}


def _build_bass():
    nc = bacc.Bacc(None, target_bir_lowering=False)

    xt_d = nc.dram_tensor("xt", [P, NCB, T], BF, kind="ExternalInput")
    wq_d = nc.dram_tensor("wq", [P, NCB, G * D], BF, kind="ExternalInput")
    wk_d = nc.dram_tensor("wk", [P, NCB, D], BF, kind="ExternalInput")
    wv_d = nc.dram_tensor("wv", [P, NCB, D], BF, kind="ExternalInput")
    wo_d = nc.dram_tensor("wo", [P, G, C], BF, kind="ExternalInput")
    cos_d = nc.dram_tensor("cosb", [P, T], BF, kind="ExternalInput")
    sin_d = nc.dram_tensor("sinb", [P, T], BF, kind="ExternalInput")
    bq_d = nc.dram_tensor("bq", [G, D, 1], F32, kind="ExternalInput")
    bk_d = nc.dram_tensor("bk", [1, D, 1], F32, kind="ExternalInput")
    bv_d = nc.dram_tensor("bv", [1, D, 1], F32, kind="ExternalInput")
    tri_d = nc.dram_tensor("tri", [P, P], BF, kind="ExternalInput")
    ident_d = nc.dram_tensor("ident", [P, P], F32, kind="ExternalInput")
    out_d = nc.dram_tensor("out", [T, C], F32, kind="ExternalOutput")

    with tile.TileContext(nc) as tc:
        with (
            tc.tile_pool(name="consts", bufs=1) as consts,
            tc.tile_pool(name="wpool", bufs=1) as wpool,
            tc.tile_pool(name="qkv", bufs=1) as qkv,
            tc.tile_pool(name="psum", bufs=2, space="PSUM") as psum,
            tc.tile_pool(name="work", bufs=3) as work,
            tc.tile_pool(name="ptp", bufs=8) as ptp,
            tc.tile_pool(name="xtp", bufs=3) as xtp,
            tc.tile_pool(name="osp", bufs=3) as osp,
        ):
            # ---- constants ----
            cos_t = consts.tile([P, T], BF)
            sin_t = consts.tile([P, T], BF)
            tri_t = consts.tile([P, P], BF)
            ident_t = consts.tile([P, P], F32)
            bq_t = consts.tile([P, G], F32)
            bk_t = consts.tile([P, 1], F32)
            bv_t = consts.tile([P, 1], F32)
            onesP_t = consts.tile([P, P], BF)
            nc.sync.dma_start(tri_t[:], tri_d[:, :])
            nc.sync.dma_start(ident_t[:], ident_d[:, :])
            for g in range(G):
                nc.sync.dma_start(bq_t[:, g : g + 1], bq_d[g])
            nc.sync.dma_start(bk_t[:], bk_d[0])
            nc.sync.dma_start(bv_t[:], bv_d[0])
            nc.vector.memset(onesP_t[:], 1.0)
            scratch_t = consts.tile([P, TC], BF)
            nc.vector.memset(scratch_t[:], 0.0)
            for _w in range(56):
                wu_ps = psum.tile([P, TC], F32, tag="st")
                nc.tensor.matmul(wu_ps[:], onesP_t[:], scratch_t[:], start=True, stop=True)

            # ---- weights (DMA in consumption order: wk, wv, wq, wo) ----
            wq_t = wpool.tile([P, NCB, G * D], BF)
            wk_t = wpool.tile([P, NCB, D], BF)
            wv_t = wpool.tile([P, NCB, D], BF)
            wo_t = wpool.tile([P, G, C], BF)
            nc.sync.dma_start(wk_t[:], wk_d[:, :, :])

            # ---- Q/K/V tensors (bf16, [d, t] layout; V as [s, d] blocks) ----
            qT = [qkv.tile([P, T], BF, name=f"qT{g}", tag=f"qT{g}") for g in range(G)]
            kT = qkv.tile([P, T], BF)
            vb = qkv.tile([P, NSB, D], BF)
            ytb = [qkv.tile([P, T], BF, name=f"yt{g}", tag=f"yt{g}") for g in range(G)]

            def project_rope(xt_c, w_ap_fn, bias_ap, out_tile, tcc):
                """psum = sum_cb W[cb].T @ xt[cb]; +bias; rotate-half RoPE -> bf16."""
                ts = slice(tcc * TC, (tcc + 1) * TC)
                ps = psum.tile([P, TC], F32, tag="proj")
                for cb in range(NCB):
                    nc.tensor.matmul(
                        ps[:], w_ap_fn(cb), xt_c[:, cb, :],
                        start=(cb == 0), stop=(cb == NCB - 1),
                    )
                qf = work.tile([P, TC], F32, tag="qf")
                nc.scalar.activation(qf[:], ps[:], AF.Identity, bias=bias_ap)
                sw = work.tile([P, TC], F32, tag="sw")
                nc.gpsimd.dma_start(sw[0:HALF, :], qf[HALF:P, :])
                nc.gpsimd.dma_start(sw[HALF:P, :], qf[0:HALF, :])
                t1 = work.tile([P, TC], BF, tag="t1")
                t2 = work.tile([P, TC], BF, tag="t2")
                nc.vector.tensor_mul(t1[:], qf[:], cos_t[:, ts])
                nc.vector.tensor_mul(t2[:], sw[:], sin_t[:, ts])
                nc.vector.tensor_add(out_tile[:, ts], t1[:], t2[:])

            def project_v(xt_c, tcc):
                ps = psum.tile([P, TC], F32, tag="proj")
                for cb in range(NCB):
                    nc.tensor.matmul(
                        ps[:], wv_t[:, cb, :], xt_c[:, cb, :],
                        start=(cb == 0), stop=(cb == NCB - 1),
                    )
                vf = work.tile([P, TC], F32, tag="qf")
                nc.scalar.activation(vf[:], ps[:], AF.Identity, bias=bv_t[:, 0:1])
                for j in range(TC // P):
                    tp = psum.tile([P, P], F32, tag="st")
                    nc.tensor.transpose(tp[:], vf[:, j * P : (j + 1) * P], ident_t[:])
                    nc.vector.tensor_copy(vb[:, tcc * (TC // P) + j, :], tp[:])

            def attn_head(g, tcc):
                """S^T attention for one head / t-chunk. AV/RS matmuls pipelined
                two s-blocks behind ST so PE never waits on the ACT exp.
                Returns a closure that emits the normalization (deferred)."""
                ts = slice(tcc * TC, (tcc + 1) * TC)
                nsb_c = 4 * tcc + 4
                yt_ps = psum.tile([P, TC], F32, tag="yt")
                rs_ps = psum.tile([P, TC], F32, tag="norm")
                pts = {}

                def emit_av(sb):
                    pt = pts.pop(sb)
                    nc.tensor.matmul(
                        yt_ps[:], vb[:, sb, :], pt[:],
                        start=(sb == 0), stop=(sb == nsb_c - 1),
                    )
                    nc.tensor.matmul(
                        rs_ps[:], onesP_t[:], pt[:],
                        start=(sb == 0), stop=(sb == nsb_c - 1),
                    )

                for sb in range(nsb_c):
                    st_ps = psum.tile([P, TC], F32, tag="st")
                    nc.tensor.matmul(
                        st_ps[:], kT[:, sb * P : (sb + 1) * P], qT[g][:, ts],
                        start=True, stop=True,
                    )
                    pt = ptp.tile([P, TC], BF, tag="pt")
                    r = sb * P - tcc * TC
                    if r >= 0:
                        if r > 0:
                            nc.vector.memset(pt[:, 0:r], 0.0)
                        nc.scalar.activation(pt[:, r:TC], st_ps[:, r:TC], AF.Exp, scale=SCALE)
                        nc.vector.tensor_mul(pt[:, r : r + P], pt[:, r : r + P], tri_t[:])
                    else:
                        nc.scalar.activation(pt[:], st_ps[:], AF.Exp, scale=SCALE)
                    pts[sb] = pt
                    if sb >= 2:
                        emit_av(sb - 2)
                for sb in range(max(0, nsb_c - 2), nsb_c):
                    emit_av(sb)

                def emit_norm():
                    rb = work.tile([P, TC], F32, tag="rb")
                    nc.vector.reciprocal(rb[:], rs_ps[:])
                    nc.vector.tensor_mul(ytb[g][:, ts], yt_ps[:], rb[:])

                return emit_norm

            def emit_wo_tb(tcc, tb, split=False):
                o_sb = osp.tile([P, C], F32, tag="osb")
                for cc in range(NTC):
                    o_ps = psum.tile([P, TC], F32, tag="proj")
                    for g in range(G):
                        nc.tensor.matmul(
                            o_ps[:],
                            ytb[g][:, tb * P : (tb + 1) * P],
                            wo_t[:, g, cc * TC : (cc + 1) * TC],
                            start=(g == 0), stop=(g == G - 1),
                        )
                    nc.scalar.activation(o_sb[:, cc * TC : (cc + 1) * TC], o_ps[:], AF.Copy)
                    if split:
                        nc.sync.dma_start(
                            out_d[tb * P : (tb + 1) * P, cc * TC : (cc + 1) * TC],
                            o_sb[:, cc * TC : (cc + 1) * TC])
                if not split:
                    nc.sync.dma_start(out_d[tb * P : (tb + 1) * P, :], o_sb[:])

            def load_xt(tcc):
                ts = slice(tcc * TC, (tcc + 1) * TC)
                xt_c = xtp.tile([P, NCB, TC], BF, tag="xt")
                for j in range(4):
                    nc.sync.dma_start(xt_c[:, 4 * j : 4 * j + 4, :],
                                      xt_d[:, 4 * j : 4 * j + 4, ts])
                return xt_c

            def proj_chunks(tcc, xt_c):
                fns = [
                    lambda: project_rope(xt_c, lambda cb: wk_t[:, cb, :],
                                         bk_t[:, 0:1], kT, tcc),
                    lambda: project_v(xt_c, tcc),
                ]
                for g in range(G):
                    fns.append(lambda g=g: project_rope(
                        xt_c,
                        lambda cb: wq_t[:, cb, g * D : (g + 1) * D],
                        bq_t[:, g : g + 1], qT[g], tcc,
                    ))
                return fns

            pending_norm = None
            # prologue: t-chunk 0 input DMA + weight DMAs + projections
            xt_c = load_xt(0)
            nc.sync.dma_start(cos_t[:], cos_d[:, :])
            nc.sync.dma_start(sin_t[:], sin_d[:, :])
            nc.sync.dma_start(wv_t[:], wv_d[:, :, :])
            nc.sync.dma_start(wq_t[:, 0:8, :], wq_d[:, 0:8, :])
            nc.sync.dma_start(wq_t[:, 8:16, :], wq_d[:, 8:16, :])
            for f in proj_chunks(0, xt_c):
                f()
            for tcc in range(NTC):
                if tcc == 0:
                    nc.sync.dma_start(wo_t[:, 0:2, :], wo_d[:, 0:2, :])
                    nc.sync.dma_start(wo_t[:, 2:4, :], wo_d[:, 2:4, :])
                chunks_next = []
                if tcc + 1 < NTC:
                    xt_next = load_xt(tcc + 1)
                    chunks_next = proj_chunks(tcc + 1, xt_next)
                for g in range(G):
                    for _ in range(2):
                        if chunks_next:
                            chunks_next.pop(0)()
                    ncl = attn_head(g, tcc)
                    if pending_norm is not None:
                        pending_norm()
                    pending_norm = ncl
                    if tcc > 0:
                        emit_wo_tb(tcc - 1, 4 * (tcc - 1) + g)
            pending_norm()
            for tb in range(4 * (NTC - 1), 4 * NTC):
                emit_wo_tb(NTC - 1, tb, split=True)
    nc.compile()
    return nc


def _host_tables():
    perm = np.concatenate([np.arange(0, D, 2), np.arange(1, D, 2)])
    inv_freq = 1.0 / (THETA ** (np.arange(0, D, 2, dtype=np.float32) / D))
    t_idx = np.arange(T, dtype=np.float32)
    ang = t_idx[:, None] * inv_freq[None, :]          # [T, 64]
    cos_half = np.cos(ang).astype(np.float32).T       # [64, T]
    sin_half = np.sin(ang).astype(np.float32).T
    cos_b = np.concatenate([cos_half, cos_half], axis=0)       # [128, T]
    sin_b = np.concatenate([-sin_half, sin_half], axis=0)      # sign baked
    si = np.arange(P)[:, None]
    tj = np.arange(P)[None, :]
    tri = (si <= tj).astype(BF16)                      # [s, t] upper-tri incl diag
    ident = np.eye(P, dtype=np.float32)
    return perm, np.ascontiguousarray(cos_b), np.ascontiguousarray(sin_b), tri, ident


def kernel(x, Wq, bq, Wk, bk, Wv, bv, Wo, bo):
    global last_run_info
    if "nc" not in _cached:
        _cached["nc"] = _build_bass()
    nc = _cached["nc"]

    x = np.asarray(x, np.float32)
    Wq = np.asarray(Wq, np.float32)
    Wk = np.asarray(Wk, np.float32)
    Wv = np.asarray(Wv, np.float32)
    Wo = np.asarray(Wo, np.float32)
    bq = np.asarray(bq, np.float32)
    bk = np.asarray(bk, np.float32)
    bv = np.asarray(bv, np.float32)
    bo = np.asarray(bo, np.float32)

    perm, cos_b, sin_b, tri, ident = _host_tables()
    cos_b = cos_b.astype(BF16)
    sin_b = sin_b.astype(BF16)

    in_maps = []
    for core in range(NCORES):
        b, kvh = divmod(core, KVH)
        xt = np.ascontiguousarray(
            x[b].T.astype(BF16).reshape(NCB, P, T).transpose(1, 0, 2))
        qcols = np.arange(kvh * G * D, (kvh + 1) * G * D)
        wq_s = Wq[:, qcols].reshape(C, G, D)[:, :, perm].reshape(C, G * D)
        wq_s = np.ascontiguousarray(
            wq_s.astype(BF16).reshape(NCB, P, G * D).transpose(1, 0, 2))
        wk_s = np.ascontiguousarray(
            Wk[:, kvh * D : (kvh + 1) * D][:, perm].astype(BF16).reshape(NCB, P, D).transpose(1, 0, 2))
        wv_s = np.ascontiguousarray(
            Wv[:, kvh * D : (kvh + 1) * D].astype(BF16).reshape(NCB, P, D).transpose(1, 0, 2))
        wo_s = np.ascontiguousarray(
            Wo[kvh * G * D : (kvh + 1) * G * D, :].astype(BF16).reshape(G, P, C).transpose(1, 0, 2))
        bq_s = bq[qcols].reshape(G, D)[:, perm].reshape(G, D, 1).astype(np.float32)
        bk_s = bk[kvh * D : (kvh + 1) * D][perm].reshape(1, D, 1).astype(np.float32)
        bv_s = bv[kvh * D : (kvh + 1) * D].reshape(1, D, 1).astype(np.float32)
        in_maps.append({
            "xt": xt, "wq": np.ascontiguousarray(wq_s),
            "wk": np.ascontiguousarray(wk_s), "wv": np.ascontiguousarray(wv_s),
            "wo": np.ascontiguousarray(wo_s),
            "cosb": cos_b, "sinb": sin_b,
            "bq": np.ascontiguousarray(bq_s), "bk": bk_s, "bv": bv_s,
            "tri": np.ascontiguousarray(tri), "ident": ident,
        })

    try:
        res = run_bass_kernel_spmd(nc, in_maps, core_ids=list(range(NCORES)))
    except ModuleNotFoundError:
        # tracing requested but profiling hooks unavailable: run without trace
        os.environ["BASS_NEVER_TRACE"] = "1"
        res = run_bass_kernel_spmd(nc, in_maps, core_ids=list(range(NCORES)))
    last_run_info = {
        "exec_time_ns": res.exec_time_ns,
        "mean_exec_time_ns": res.mean_exec_time_ns,
        "profile_json": res.profile_json,
    }

    out = np.zeros((B, T, C), np.float32)
    for core in range(NCORES):
        b = core // KVH
        out[b] += res.results[core]["out"].astype(np.float32)
    out += bo[None, None, :]
    return out
